# revision 14
# baseline (speedup 1.0000x reference)
"""Dense transformer block (rmsnorm+causal attention+rope / rmsnorm+SwiGLU) on 8 TRN2 cores.

Sharding v2:
  core j: heads {2j, 2j+1} x BOTH batches (4 head-instances).  Every core
  therefore holds attention output destined for all 8 phase-B owners, so the
  two AllToAlls carry fully-dense payload (no cross-batch zero slots, no
  bmask, no receiver slot-pair sums).  A2A1 fires after the even-head
  instances (50% of attention), A2A2 after the odd-head instances; proj
  pass 0 overlaps A2A2's flight.
  Phase B owner of core j: batch j//4, token block j%4 (as baseline).

  Phase A: rmsnorm1 stats via ScalarE squares + DVE ct-accumulation + one
  ones-matmul per chunk; q/k/v matmuls run on RAW x (rstd folded into the
  rope tables for q/k, transposed per-row scale for v) so the PE never
  waits on the stats chain.  Causal attention keeps q/k/v SBUF-resident;
  softmax denominators are DVE-accumulated (bf16 eacc, interleaved with
  exp) + two 512-col ones-matmuls emitted after the AV pass.
  Phase B runs fully transposed: proj y^T accumulates [C, TQ], residual
  read straight from x_t (bf16), rmsnorm2 + SwiGLU with a transposed w3
  pass; output is [C, TQ] per core, transposed on host.

Matmul operands are bf16 (weights and x pre-cast on host, w_norm folded into
weight rows); statistics, softmax sums, residual accumulation and PSUM stay
fp32.
"""

import numpy as np
import ml_dtypes

import concourse.bass as bass
import concourse.mybir as mybir
import concourse.tile as tile
from concourse import bacc
from concourse import bass_utils
from concourse.masks import make_identity

AF = mybir.ActivationFunctionType
ALU = mybir.AluOpType
F32 = mybir.dt.float32
BF16 = mybir.dt.bfloat16
MMDT = BF16
NP_MMDT = ml_dtypes.bfloat16

P = 128
T = 2048
C = 2048
D = 128
H = 16
HPC = 2          # heads per core
NB = 2           # batches (all processed on every core)
HID = 5632
HID_T = HID // P  # 44 hid tiles
TQ = 512         # A2A / output col-block granularity
TQA = 1024       # attention query-chunk
EPS = 1e-6
ROPE_BASE = 10000.0
CT = C // P      # 16 contraction tiles
HD2 = D // 2


def _build():
    nc = bacc.Bacc(None, target_bir_lowering=False, num_devices=8)

    # ---- kernel I/O ----
    x_t = nc.dram_tensor("x_t", [C, NB * T], MMDT, kind="ExternalInput")
    wq = nc.dram_tensor("wq", [HPC, P, CT * P], MMDT, kind="ExternalInput")
    wk = nc.dram_tensor("wk", [HPC, P, CT * P], MMDT, kind="ExternalInput")
    wv = nc.dram_tensor("wv", [P, CT, HPC * D], MMDT, kind="ExternalInput")
    wpe_r = nc.dram_tensor("wpe_r", [2, CT, P, 8 * P], MMDT, kind="ExternalInput")
    w1t = nc.dram_tensor("w1t", [HID_T, P, CT * P], MMDT, kind="ExternalInput")
    w2t = nc.dram_tensor("w2t", [HID_T, P, CT * P], MMDT, kind="ExternalInput")
    w3r = nc.dram_tensor("w3r", [CT, P, HID_T * P], MMDT, kind="ExternalInput")
    rope_t = nc.dram_tensor("rope_t", [D, T], F32, kind="ExternalInput")
    tri = nc.dram_tensor("tri", [P, P], MMDT, kind="ExternalInput")
    x_res = nc.dram_tensor("x_res", [C, TQ], MMDT, kind="ExternalInput")
    out = nc.dram_tensor("out", [C, TQ], F32, kind="ExternalOutput")

    inv_sqrt_d = 1.0 / float(np.sqrt(D))
    GROUPS = [[0, 1, 2, 3, 4, 5, 6, 7]]

    with tile.TileContext(nc) as tc:
        with (
            tc.tile_pool(name="const", bufs=1) as const,
            tc.tile_pool(name="dram", bufs=1, space="DRAM") as dram,
        ):
            # ---- constants ----
            ones_f = const.tile([P, 1], F32)
            nc.vector.memset(ones_f, 1.0)
            ones_r = const.tile([P, 1], MMDT)
            nc.vector.tensor_copy(out=ones_r, in_=ones_f)
            eps_sb = const.tile([P, 1], F32)
            nc.vector.memset(eps_sb, EPS)
            ident_f = const.tile([P, P], F32)
            make_identity(nc, ident_f)
            rope_sb = const.tile([D, T], F32)
            tri_sb = const.tile([P, P], MMDT)

            # ---- DRAM scratch for collectives ----
            a2a1_in = dram.tile([8, P, TQ], MMDT)
            a2a1_out = dram.tile([8, P, TQ], MMDT)
            a2a2_in = dram.tile([8, P, TQ], MMDT)
            a2a2_out = dram.tile([8, P, TQ], MMDT)

            # ---- persistent SBUF across phase A (q/k/v resident) ----
            # instance index: inst = 2*m + b  (m: head-in-pair, b: batch)
            qkvp_ctx = tc.tile_pool(name="qkvp", bufs=1)
            qkvp = qkvp_ctx.__enter__()
            qT_sb = qkvp.tile([P, 2 * HPC, T], MMDT, tag="qT_sb", bufs=1)
            kT_sb = qkvp.tile([P, 2 * HPC, T], MMDT, tag="kT_sb", bufs=1)
            v_sb = qkvp.tile([P, NB * T // P, HPC * D], MMDT, tag="v_sb", bufs=1)

            # ================= Phase A1+A2: rmsnorm1 + QKV (chunked) ============
            with (
                tc.tile_pool(name="p12", bufs=2) as p12,
                tc.tile_pool(name="p12psum", bufs=2, space="PSUM") as pp12,
            ):
                CHUNKS = [256, 256, 512, 512, 512]  # per batch (sums to T)
                first_xt = p12.tile([P, CT, CHUNKS[0]], MMDT, tag="xt256", bufs=2)
                nc.sync.dma_start(
                    out=first_xt,
                    in_=x_t[:, 0 : CHUNKS[0]].rearrange("(ct p) t -> p ct t", p=P),
                )
                wq_sb = p12.tile([P, HPC, CT, P], MMDT, tag="wq_sb", bufs=1)
                wk_sb = p12.tile([P, HPC, CT, P], MMDT, tag="wk_sb", bufs=1)
                for m_ in range(HPC):
                    nc.sync.dma_start(out=wq_sb[:, m_], in_=wq[m_])
                for m_ in range(HPC):
                    nc.sync.dma_start(out=wk_sb[:, m_], in_=wk[m_])
                wv_sb = p12.tile([P, CT, HPC * D], MMDT, tag="wv_sb", bufs=1)
                nc.sync.dma_start(out=wv_sb, in_=wv[:, :, :])
                nc.sync.dma_start(out=rope_sb, in_=rope_t[:, :])
                nc.sync.dma_start(out=tri_sb, in_=tri[:, :])

                for b in range(NB):
                    t0p = 0  # within-batch position
                    for ch, CHW in enumerate(CHUNKS):
                        t0 = b * T + t0p  # global column in x_t / v_sb
                        if b == 0 and ch == 0:
                            xt = first_xt
                        else:
                            xt = p12.tile(
                                [P, CT, CHW], MMDT, tag=f"xt{CHW}", bufs=2, name="xt"
                            )
                            nc.sync.dma_start(
                                out=xt,
                                in_=x_t[:, t0 : t0 + CHW].rearrange(
                                    "(ct p) t -> p ct t", p=P
                                ),
                            )
                        # rmsnorm stats: squares on ScalarE, ct-accum on DVE,
                        # partition-sum via one ones-matmul
                        sq = p12.tile([P, CT, CHW], MMDT, tag=f"sq{CHW}", bufs=1, name="sq")
                        nc.scalar.activation(
                            sq.rearrange("p a b -> p (a b)"),
                            xt.rearrange("p a b -> p (a b)"),
                            AF.Square,
                        )
                        sqacc = p12.tile([P, TQ], MMDT, tag="sqacc", bufs=2, name="sqacc")[:, :CHW]
                        nc.vector.tensor_tensor(
                            out=sqacc, in0=sq[:, 0, :], in1=sq[:, 1, :], op=ALU.add
                        )
                        for ct in range(2, CT):
                            nc.vector.tensor_tensor(
                                out=sqacc, in0=sqacc, in1=sq[:, ct, :], op=ALU.add
                            )
                        ssum = pp12.tile([1, TQ], F32, tag="ssum", bufs=1, name="ssum")[:, :CHW]
                        nc.tensor.matmul(ssum, ones_r, sqacc, start=True, stop=True)
                        srow = p12.tile([1, TQ], F32, tag="srow", bufs=2, name="srow")[:, :CHW]
                        nc.scalar.activation(
                            srow, ssum, AF.Sqrt, bias=eps_sb[0:1, :], scale=1.0 / C
                        )
                        rstd_row = p12.tile([1, TQ], F32, tag="rstd_row", bufs=2, name="rstd_row")[:, :CHW]
                        nc.vector.reciprocal_approx_fast(out=rstd_row, in_=srow)
                        rstd_bc = p12.tile([P, TQ], F32, tag="rstd_bc", bufs=2, name="rstd_bc")[:, :CHW]
                        nc.gpsimd.partition_broadcast(rstd_bc[:], rstd_row[:])
                        # rstd folded into rope tables: rows 0:64 cos*rstd,
                        # 64:128 sin*rstd (q/k matmuls run on RAW x so the PE
                        # never waits on the stats chain)
                        cs_r = p12.tile([P, TQ], F32, tag="cs_r", bufs=2, name="cs_r")[:, :CHW]
                        nc.vector.tensor_tensor(
                            out=cs_r, in0=rope_sb[:, t0p : t0p + CHW], in1=rstd_bc,
                            op=ALU.mult,
                        )

                        # q^T / k^T with fused rope(+rstd) on eviction
                        for w_sb, dst in ((wq_sb, qT_sb), (wk_sb, kT_sb)):
                            for m in range(HPC):
                                inst = 2 * m + b
                                pq = pp12.tile([P, TQ], F32, tag="qk", bufs=3, name="pq")[:, :CHW]
                                for ct in range(CT):
                                    nc.tensor.matmul(
                                        pq,
                                        w_sb[:, m, ct, :],
                                        xt[:, ct, :],
                                        start=(ct == 0),
                                        stop=(ct == CT - 1),
                                    )
                                x1 = pq[0:HD2, :]
                                x2 = pq[HD2:P, :]
                                cosw = cs_r[0:HD2, :]
                                sinw = cs_r[HD2:P, :]
                                tm1 = p12.tile([HD2, TQ], F32, tag="tm1", bufs=2, name="tm1")[:, :CHW]
                                tm2 = p12.tile([HD2, TQ], F32, tag="tm2", bufs=2, name="tm2")[:, :CHW]
                                nc.vector.tensor_tensor(out=tm1, in0=x1, in1=cosw, op=ALU.mult)
                                nc.vector.tensor_tensor(out=tm2, in0=x2, in1=sinw, op=ALU.mult)
                                nc.vector.tensor_tensor(
                                    out=dst[0:HD2, inst, t0p : t0p + CHW],
                                    in0=tm1,
                                    in1=tm2,
                                    op=ALU.subtract,
                                )
                                nc.vector.tensor_tensor(out=tm1, in0=x1, in1=sinw, op=ALU.mult)
                                nc.vector.tensor_tensor(out=tm2, in0=x2, in1=cosw, op=ALU.mult)
                                nc.vector.tensor_tensor(
                                    out=dst[HD2:P, inst, t0p : t0p + CHW],
                                    in0=tm1,
                                    in1=tm2,
                                    op=ALU.add,
                                )

                        # v in row layout [t, 2*D]; per-row rstd via PE-transposed col
                        for rt in range(CHW // P):
                            trp = pp12.tile([P, P], F32, tag="trp", bufs=2)
                            nc.tensor.transpose(
                                trp, rstd_bc[:, rt * P : (rt + 1) * P], ident_f
                            )
                            rstd_col = p12.tile([P, 1], F32, tag="rstd_col", bufs=2)
                            nc.vector.tensor_copy(out=rstd_col, in_=trp[:, 0:1])
                            pv = pp12.tile([P, HPC * D], F32, tag="v", bufs=2)
                            for ct in range(CT):
                                nc.tensor.matmul(
                                    pv,
                                    xt[:, ct, rt * P : (rt + 1) * P],
                                    wv_sb[:, ct, :],
                                    start=(ct == 0),
                                    stop=(ct == CT - 1),
                                )
                            nc.vector.tensor_scalar(
                                out=v_sb[:, t0 // P + rt, :],
                                in0=pv,
                                scalar1=rstd_col,
                                scalar2=None,
                                op0=ALU.mult,
                            )
                        t0p += CHW

            # ================= Phase A3: causal attention (+ A2As) ==============
            with (
                tc.tile_pool(name="att", bufs=2) as att,
                tc.tile_pool(name="attpsum", bufs=2, space="PSUM") as pat,
            ):
                for m in range(HPC):
                    a2a_in = (a2a1_in, a2a2_in)[m]
                    for b in range(NB):
                        inst = 2 * m + b
                        for q2 in range(T // TQA):
                            qb = q2 * TQA
                            o_ps = pat.tile([P, TQA], F32, tag="o", bufs=1)
                            es = []
                            # full key blocks
                            for kb in range(8 * q2):
                                st = pat.tile([P, TQA], F32, tag="st", bufs=2)
                                for i in range(2):
                                    nc.tensor.matmul(
                                        st[:, i * TQ : (i + 1) * TQ],
                                        kT_sb[:, inst, kb * P : (kb + 1) * P],
                                        qT_sb[:, inst, qb + i * TQ : qb + (i + 1) * TQ],
                                        start=True,
                                        stop=True,
                                    )
                                e = att.tile([P, TQA], MMDT, tag="e", bufs=18)
                                nc.scalar.activation(e, st, AF.Exp, scale=inv_sqrt_d)
                                if not es:
                                    eacc = att.tile([P, TQA], MMDT, tag="eacc", bufs=2)
                                    nc.vector.tensor_copy(out=eacc, in_=e)
                                else:
                                    nc.vector.tensor_tensor(
                                        out=eacc, in0=eacc, in1=e, op=ALU.add
                                    )
                                es.append((kb, 0, e))
                            # diagonal blocks (r = 0..7), masked region trimmed
                            for r in range(8):
                                kb = 8 * q2 + r
                                q0 = r * P
                                st = pat.tile([P, TQA], F32, tag="st", bufs=2)
                                if q0 < TQ:
                                    nc.tensor.matmul(
                                        st[:, q0:TQ],
                                        kT_sb[:, inst, kb * P : (kb + 1) * P],
                                        qT_sb[:, inst, qb + q0 : qb + TQ],
                                        start=True,
                                        stop=True,
                                    )
                                    nc.tensor.matmul(
                                        st[:, TQ:TQA],
                                        kT_sb[:, inst, kb * P : (kb + 1) * P],
                                        qT_sb[:, inst, qb + TQ : qb + TQA],
                                        start=True,
                                        stop=True,
                                    )
                                else:
                                    nc.tensor.matmul(
                                        st[:, q0:TQA],
                                        kT_sb[:, inst, kb * P : (kb + 1) * P],
                                        qT_sb[:, inst, qb + q0 : qb + TQA],
                                        start=True,
                                        stop=True,
                                    )
                                e = att.tile([P, TQA], MMDT, tag="e", bufs=18)
                                nc.scalar.activation(
                                    e[:, q0:TQA], st[:, q0:TQA], AF.Exp, scale=inv_sqrt_d
                                )
                                nc.vector.tensor_tensor(
                                    out=e[:, q0 : q0 + P],
                                    in0=e[:, q0 : q0 + P],
                                    in1=tri_sb,
                                    op=ALU.mult,
                                )
                                if not es:
                                    eacc = att.tile([P, TQA], MMDT, tag="eacc", bufs=2)
                                    nc.vector.tensor_copy(out=eacc, in_=e)
                                else:
                                    nc.vector.tensor_tensor(
                                        out=eacc[:, q0:TQA],
                                        in0=eacc[:, q0:TQA],
                                        in1=e[:, q0:TQA],
                                        op=ALU.add,
                                    )
                                es.append((kb, q0, e))
                            n_items = len(es)
                            # last writer of bank0 (cols 0:TQ) is the r=3 diag item;
                            # last writer of bank1 is the final (r=7) item
                            b0_last = n_items - 5
                            # AV pass
                            for idx, (kb, q0, e) in enumerate(es):
                                first = idx == 0
                                if q0 < TQ:
                                    nc.tensor.matmul(
                                        o_ps[:, q0:TQ],
                                        v_sb[:, b * (T // P) + kb, m * D : (m + 1) * D],
                                        e[:, q0:TQ],
                                        start=first, stop=(idx == b0_last),
                                    )
                                    nc.tensor.matmul(
                                        o_ps[:, TQ:TQA],
                                        v_sb[:, b * (T // P) + kb, m * D : (m + 1) * D],
                                        e[:, TQ:TQA],
                                        start=first, stop=(idx == n_items - 1),
                                    )
                                else:
                                    nc.tensor.matmul(
                                        o_ps[:, q0:TQA],
                                        v_sb[:, b * (T // P) + kb, m * D : (m + 1) * D],
                                        e[:, q0:TQA],
                                        start=first, stop=(idx == n_items - 1),
                                    )
                            # denominator matmuls after AV so the PE never waits
                            # on the DVE eacc chain
                            l_ps = pat.tile([1, TQA], F32, tag="l", bufs=1)
                            nc.tensor.matmul(
                                l_ps[:, 0:TQ], ones_r, eacc[:, 0:TQ],
                                start=True, stop=True,
                            )
                            nc.tensor.matmul(
                                l_ps[:, TQ:TQA], ones_r, eacc[:, TQ:TQA],
                                start=True, stop=True,
                            )
                            l_inv = att.tile([1, TQA], F32, tag="l_inv", bufs=2)
                            nc.vector.reciprocal_approx_fast(out=l_inv, in_=l_ps)
                            l_bc = att.tile([P, TQA], F32, tag="l_bc", bufs=2)
                            nc.gpsimd.partition_broadcast(l_bc[:], l_inv[:])
                            oT = att.tile([P, TQA], MMDT, tag="oT", bufs=2)
                            nc.vector.tensor_tensor(out=oT, in0=o_ps, in1=l_bc, op=ALU.mult)
                            for i in range(2):
                                qc = 2 * q2 + i  # phase-B token block 0..3
                                nc.sync.dma_start(
                                    out=a2a_in[b * 4 + qc, :, :],
                                    in_=oT[:, i * TQ : (i + 1) * TQ],
                                )
                    if m == 0:
                        nc.gpsimd.collective_compute(
                            "AllToAll",
                            ALU.bypass,
                            replica_groups=GROUPS,
                            ins=[a2a1_in.opt()],
                            outs=[a2a1_out.opt()],
                        )
            qkvp_ctx.__exit__(None, None, None)

            # A2A2 emitted outside the attention pool so its completion doesn't
            # gate the pool-close barrier; it overlaps proj pass 0.
            nc.gpsimd.collective_compute(
                "AllToAll",
                ALU.bypass,
                replica_groups=GROUPS,
                ins=[a2a2_in.opt()],
                outs=[a2a2_out.opt()],
            )

            # ---- persistent SBUF through phase B ----
            bper_ctx = tc.tile_pool(name="bper", bufs=1)
            bper = bper_ctx.__enter__()
            xmidT = bper.tile([P, CT, TQ], F32, tag="xmidT", bufs=1)
            h2T = bper.tile([P, CT, TQ], MMDT, tag="h2T", bufs=1)

            # ========== Phase B1: proj^T + residual + rmsnorm2 (transposed) ======
            with (
                tc.tile_pool(name="proj", bufs=2) as prj,
                tc.tile_pool(name="projpsum", bufs=2, space="PSUM") as ppj,
            ):
                lp0 = prj.tile([P, 8, TQ], MMDT, tag="lp0", bufs=1)
                lp1 = prj.tile([P, 8, TQ], MMDT, tag="lp1", bufs=1)
                wpe_first = prj.tile([P, 8, P], MMDT, tag="wpe_first", bufs=1)
                nc.sync.dma_start(out=wpe_first, in_=wpe_r[0, 0, :, 0 : 8 * P])
                # lp0[p, s, t] = a2a1_out[s, p, t]: slot s = head 2s, one DMA.
                nc.sync.dma_start(
                    out=lp0, in_=a2a1_out.rearrange("s p t -> p s t")
                )
                # pass 0: even heads (a2a1), into xmidT
                for ct in range(CT):
                    if ct == 0:
                        wpe_sb = wpe_first
                    else:
                        wpe_sb = prj.tile([P, 8, P], MMDT, tag="wpe_sb", bufs=3)
                        nc.sync.dma_start(out=wpe_sb, in_=wpe_r[0, ct, :, 0 : 8 * P])
                    yps = ppj.tile([P, TQ], F32, tag="y", bufs=4)
                    for blk in range(8):
                        nc.tensor.matmul(
                            yps,
                            wpe_sb[:, blk, :],
                            lp0[:, blk, :],
                            start=(blk == 0),
                            stop=(blk == 7),
                        )
                    nc.scalar.copy(out=xmidT[:, ct, :], in_=yps)
                # residual slice of x (bf16, per-core input)
                xres = prj.tile([P, CT, TQ], MMDT, tag="xres", bufs=1)
                nc.sync.dma_start(
                    out=xres,
                    in_=x_res[:, :].rearrange("(ct p) t -> p ct t", p=P),
                )
                # pass 1: odd heads (a2a2) + residual, rmsnorm2 stats per ct
                nc.sync.dma_start(
                    out=lp1, in_=a2a2_out.rearrange("s p t -> p s t")
                )
                sq2 = bper.tile([P, CT, TQ], MMDT, tag="sq2", bufs=1)
                sq2acc = bper.tile([P, TQ], MMDT, tag="sq2acc", bufs=1)
                for ct in range(CT):
                    wpe_sb1 = prj.tile([P, 8, P], MMDT, tag="wpe_sb1", bufs=3)
                    nc.sync.dma_start(
                        out=wpe_sb1, in_=wpe_r[1, ct, :, 0 : 8 * P]
                    )
                    yps = ppj.tile([P, TQ], F32, tag="y", bufs=4)
                    for blk in range(8):
                        nc.tensor.matmul(
                            yps,
                            wpe_sb1[:, blk, :],
                            lp1[:, blk, :],
                            start=(blk == 0),
                            stop=(blk == 7),
                        )
                    t1 = prj.tile([P, TQ], F32, tag="t1", bufs=3)
                    nc.vector.tensor_tensor(
                        out=t1, in0=yps, in1=xmidT[:, ct, :], op=ALU.add
                    )
                    nc.vector.tensor_tensor(
                        out=xmidT[:, ct, :], in0=t1, in1=xres[:, ct, :], op=ALU.add
                    )
                    nc.scalar.activation(
                        sq2[:, ct, :], xmidT[:, ct, :], AF.Square
                    )
                    if ct == 1:
                        nc.vector.tensor_tensor(
                            out=sq2acc, in0=sq2[:, 0, :], in1=sq2[:, 1, :], op=ALU.add
                        )
                    elif ct > 1:
                        nc.vector.tensor_tensor(
                            out=sq2acc, in0=sq2acc, in1=sq2[:, ct, :], op=ALU.add
                        )
                ssum2 = ppj.tile([1, TQ], F32, tag="ssum2", bufs=1)
                nc.tensor.matmul(ssum2, ones_r, sq2acc, start=True, stop=True)
                srow2 = bper.tile([1, TQ], F32, tag="srow2", bufs=1)
                nc.scalar.activation(
                    srow2, ssum2, AF.Sqrt, bias=eps_sb[0:1, :], scale=1.0 / C
                )
                rstd2 = bper.tile([1, TQ], F32, tag="rstd2", bufs=1)
                nc.vector.reciprocal_approx_fast(out=rstd2, in_=srow2)
                rstd2_bc = bper.tile([P, TQ], F32, tag="rstd2_bc", bufs=1)
                nc.gpsimd.partition_broadcast(rstd2_bc[:], rstd2[:])
                for ct in range(CT):
                    nc.vector.tensor_tensor(
                        out=h2T[:, ct, :], in0=xmidT[:, ct, :], in1=rstd2_bc, op=ALU.mult
                    )

            # ================= Phase B2: SwiGLU (transposed w3 pass) =============
            with (
                tc.tile_pool(name="mlp", bufs=2) as mlp,
                tc.tile_pool(name="mlppsum", bufs=2, space="PSUM") as pml,
            ):
                uT = mlp.tile([P, HID_T, TQ], MMDT, tag="uT", bufs=1)
                for ht in range(HID_T):
                    w1_sb = mlp.tile([P, CT, P], MMDT, tag="w1_sb", bufs=3)
                    nc.sync.dma_start(out=w1_sb, in_=w1t[ht])
                    w2_sb = mlp.tile([P, CT, P], MMDT, tag="w2_sb", bufs=3)
                    nc.sync.dma_start(out=w2_sb, in_=w2t[ht])
                    g1 = pml.tile([P, TQ], F32, tag="g1", bufs=2)
                    g2 = pml.tile([P, TQ], F32, tag="g2", bufs=2)
                    for ct in range(CT):
                        nc.tensor.matmul(
                            g1, w1_sb[:, ct, :], h2T[:, ct, :],
                            start=(ct == 0), stop=(ct == CT - 1),
                        )
                    for ct in range(CT):
                        nc.tensor.matmul(
                            g2, w2_sb[:, ct, :], h2T[:, ct, :],
                            start=(ct == 0), stop=(ct == CT - 1),
                        )
                    sil = mlp.tile([P, TQ], F32, tag="sil", bufs=3)
                    nc.scalar.activation(sil, g1, AF.Silu)
                    nc.vector.tensor_tensor(
                        out=uT[:, ht, :], in0=g2, in1=sil, op=ALU.mult
                    )
                # y3^T: stationary w3 blocks, moving uT; accumulate 44 ht per ct
                for ct in range(CT):
                    w3_sb = mlp.tile([P, HID_T, P], MMDT, tag="w3_sb", bufs=2)
                    nc.sync.dma_start(out=w3_sb, in_=w3r[ct])
                    y3 = pml.tile([P, TQ], F32, tag="y3", bufs=2)
                    for ht in range(HID_T):
                        nc.tensor.matmul(
                            y3, w3_sb[:, ht, :], uT[:, ht, :],
                            start=(ht == 0), stop=(ht == HID_T - 1),
                        )
                    ofin = mlp.tile([P, TQ], F32, tag="ofin", bufs=3)
                    nc.vector.tensor_tensor(
                        out=ofin, in0=y3, in1=xmidT[:, ct, :], op=ALU.add
                    )
                    nc.sync.dma_start(out=out[ct * P : (ct + 1) * P, :], in_=ofin)
            bper_ctx.__exit__(None, None, None)

    nc.compile()
    return nc


_NC_CACHE = None


def _get_nc():
    global _NC_CACHE
    if _NC_CACHE is None:
        _NC_CACHE = _build()
    return _NC_CACHE


def _host_inputs(x, w_norm1, w_qkv, w_proj, w_norm2, w1, w2, w3):
    x = np.asarray(x, dtype=np.float32)
    w_qkv = np.asarray(w_qkv, dtype=np.float32)
    w_proj = np.asarray(w_proj, dtype=np.float32)
    w_norm1 = np.asarray(w_norm1, dtype=np.float32)
    w_norm2 = np.asarray(w_norm2, dtype=np.float32)
    w1 = np.asarray(w1, dtype=np.float32)
    w2 = np.asarray(w2, dtype=np.float32)
    w3 = np.asarray(w3, dtype=np.float32)

    half = D // 2
    inv_freq = 1.0 / (ROPE_BASE ** (np.arange(half, dtype=np.float32) / half))
    pos = np.arange(T, dtype=np.float32)
    freqs = pos[:, None] * inv_freq[None, :]
    rope_tab = np.ascontiguousarray(
        np.concatenate([np.cos(freqs).T, np.sin(freqs).T], axis=0).astype(np.float32)
    )

    ql = np.arange(P)[None, :]
    kv = np.arange(P)[:, None]
    tri = (ql >= kv).astype(NP_MMDT)

    # fold w_norm into weight rows (h @ W == (x*rstd) @ (diag(wn) W))
    w_qkv_n = w_qkv * w_norm1[:, None]
    w1_n = w1 * w_norm2[:, None]
    w2_n = w2 * w_norm2[:, None]

    # [HID_T, P, CT*P]: w1t[ht, p, ct*P + d] = w1_n[ct*P + p, ht*P + d]
    w1t = np.ascontiguousarray(
        w1_n.reshape(CT, P, HID_T, P).transpose(2, 1, 0, 3).reshape(HID_T, P, C)
    ).astype(NP_MMDT)
    w2t = np.ascontiguousarray(
        w2_n.reshape(CT, P, HID_T, P).transpose(2, 1, 0, 3).reshape(HID_T, P, C)
    ).astype(NP_MMDT)
    # [CT, P, HID_T*P]: w3r[ct, p, ht*P + d] = w3[ht*P + p, ct*P + d]
    w3r_h = np.ascontiguousarray(
        w3.reshape(HID_T, P, CT, P).transpose(2, 1, 0, 3).reshape(CT, P, HID)
    ).astype(NP_MMDT)

    # [P, CT, cols]: wqkv_r[p, ct, d] = w_qkv_n[ct*P + p, d]
    wqkv_r = np.ascontiguousarray(
        w_qkv_n.reshape(CT, P, 3 * C).transpose(1, 0, 2)
    ).astype(NP_MMDT)

    # wpe: [2, CT, P, 8*P].  Pass p block s = w_proj rows of head (2s + p)
    # (receiver slot s carries head 2s for pass 0, head 2s+1 for pass 1).
    wpe_full = np.empty((2, 8, P, C), dtype=np.float32)
    for s_ in range(8):
        wpe_full[0, s_] = w_proj[(2 * s_) * P : (2 * s_ + 1) * P, :]
        wpe_full[1, s_] = w_proj[(2 * s_ + 1) * P : (2 * s_ + 2) * P, :]
    wpe_r_h = np.ascontiguousarray(
        wpe_full.reshape(2, 8, P, CT, P).transpose(0, 3, 2, 1, 4).reshape(2, CT, P, 8 * P)
    ).astype(NP_MMDT)

    # x_t: [C, 2T] both batches, bf16 (shared across cores)
    x_t_h = np.ascontiguousarray(
        np.concatenate([x[0].T, x[1].T], axis=1)
    ).astype(NP_MMDT)

    in_maps = []
    for j in range(8):
        b, hg = j // 4, j % 4
        col0 = 2 * j * D  # first head of this core's pair
        in_maps.append(
            {
                "x_t": x_t_h,
                "wq": np.ascontiguousarray(
                    wqkv_r[:, :, col0 : col0 + HPC * D]
                    .reshape(P, CT, HPC, P)
                    .transpose(2, 0, 1, 3)
                    .reshape(HPC, P, C)
                ),
                "wk": np.ascontiguousarray(
                    wqkv_r[:, :, C + col0 : C + col0 + HPC * D]
                    .reshape(P, CT, HPC, P)
                    .transpose(2, 0, 1, 3)
                    .reshape(HPC, P, C)
                ),
                "wv": np.ascontiguousarray(
                    wqkv_r[:, :, 2 * C + col0 : 2 * C + col0 + HPC * D]
                ),
                "wpe_r": wpe_r_h,
                "w1t": w1t,
                "w2t": w2t,
                "w3r": w3r_h,
                "rope_t": rope_tab,
                "tri": tri,
                "x_res": np.ascontiguousarray(
                    x_t_h[:, b * T + hg * TQ : b * T + (hg + 1) * TQ]
                ),
            }
        )
    return in_maps


def kernel(x, w_norm1, w_qkv, w_proj, w_norm2, w1, w2, w3, _trace=False, _tmpdir=None):
    nc = _get_nc()
    in_maps = _host_inputs(x, w_norm1, w_qkv, w_proj, w_norm2, w1, w2, w3)
    kwargs = {}
    if _trace:
        kwargs = {"trace": True, "tmpdir": _tmpdir}
    res = bass_utils.run_bass_kernel_spmd(
        nc, in_maps, core_ids=list(range(8)), **kwargs
    )
    out = np.empty((2, T, C), dtype=np.float32)
    for j in range(8):
        out[j // 4, (j % 4) * TQ : (j % 4 + 1) * TQ, :] = res.results[j]["out"].T
    kernel._last_exec_time_ns = res.exec_time_ns
    return out


# revision 18
# speedup vs baseline: 1.0724x; 1.0724x over previous
"""Dense transformer block (rmsnorm+causal attention+rope / rmsnorm+SwiGLU) on 8 TRN2 cores.

Sharding v2:
  core j: heads {2j, 2j+1} x BOTH batches (4 head-instances).  Every core
  therefore holds attention output destined for all 8 phase-B owners, so the
  two AllToAlls carry fully-dense payload (no cross-batch zero slots, no
  bmask, no receiver slot-pair sums).  A2A1 fires after the even-head
  instances (50% of attention), A2A2 after the odd-head instances; proj
  pass 0 overlaps A2A2's flight.
  Phase B owner of core j: batch j//4, token block j%4 (as baseline).

  Phase A: rmsnorm1 stats via ScalarE squares + DVE ct-accumulation + one
  ones-matmul per chunk; q/k/v matmuls run on RAW x (rstd folded into the
  rope tables for q/k, transposed per-row scale for v) so the PE never
  waits on the stats chain.  Causal attention keeps q/k/v SBUF-resident;
  softmax denominators are DVE-accumulated (bf16 eacc, interleaved with
  exp) + two 512-col ones-matmuls emitted after the AV pass.
  Phase B runs fully transposed: proj y^T accumulates [C, TQ], residual
  read straight from x_t (bf16), rmsnorm2 + SwiGLU with a transposed w3
  pass; output is [C, TQ] per core, transposed on host.

Matmul operands are bf16 (weights and x pre-cast on host, w_norm folded into
weight rows); statistics, softmax sums, residual accumulation and PSUM stay
fp32.
"""

import numpy as np
import ml_dtypes

import concourse.bass as bass
import concourse.mybir as mybir
import concourse.tile as tile
from concourse import bacc
from concourse import bass_utils
from concourse.masks import make_identity

AF = mybir.ActivationFunctionType
ALU = mybir.AluOpType
F32 = mybir.dt.float32
BF16 = mybir.dt.bfloat16
MMDT = BF16
NP_MMDT = ml_dtypes.bfloat16

P = 128
T = 2048
C = 2048
D = 128
H = 16
HPC = 2          # heads per core
NB = 2           # batches (all processed on every core)
HID = 5632
HID_T = HID // P  # 44 hid tiles
TQ = 512         # A2A / output col-block granularity
TQA = 1024       # attention query-chunk
EPS = 1e-6
ROPE_BASE = 10000.0
CT = C // P      # 16 contraction tiles
HD2 = D // 2


def _build():
    nc = bacc.Bacc(None, target_bir_lowering=False, num_devices=8)

    # ---- kernel I/O ----
    x_t = nc.dram_tensor("x_t", [C, NB * T], MMDT, kind="ExternalInput")
    wq = nc.dram_tensor("wq", [HPC, P, CT * P], MMDT, kind="ExternalInput")
    wk = nc.dram_tensor("wk", [HPC, P, CT * P], MMDT, kind="ExternalInput")
    wv = nc.dram_tensor("wv", [P, CT, HPC * D], MMDT, kind="ExternalInput")
    wpe_r = nc.dram_tensor("wpe_r", [2, CT, P, 8 * P], MMDT, kind="ExternalInput")
    w1t = nc.dram_tensor("w1t", [HID_T, P, CT * P], MMDT, kind="ExternalInput")
    w2t = nc.dram_tensor("w2t", [HID_T, P, CT * P], MMDT, kind="ExternalInput")
    w3r = nc.dram_tensor("w3r", [CT, P, HID_T * P], MMDT, kind="ExternalInput")
    rope_t = nc.dram_tensor("rope_t", [D, T], F32, kind="ExternalInput")
    tri = nc.dram_tensor("tri", [P, P], MMDT, kind="ExternalInput")
    x_res = nc.dram_tensor("x_res", [C, TQ], MMDT, kind="ExternalInput")
    out = nc.dram_tensor("out", [C, TQ], F32, kind="ExternalOutput")

    inv_sqrt_d = 1.0 / float(np.sqrt(D))
    GROUPS = [[0, 1, 2, 3, 4, 5, 6, 7]]

    with tile.TileContext(nc) as tc:
        with (
            tc.tile_pool(name="const", bufs=1) as const,
            tc.tile_pool(name="dram", bufs=1, space="DRAM") as dram,
        ):
            # ---- constants ----
            ones_f = const.tile([P, 1], F32)
            nc.vector.memset(ones_f, 1.0)
            ones_r = const.tile([P, 1], MMDT)
            nc.vector.tensor_copy(out=ones_r, in_=ones_f)
            eps_sb = const.tile([P, 1], F32)
            nc.vector.memset(eps_sb, EPS)
            ident_f = const.tile([P, P], F32)
            make_identity(nc, ident_f)
            rope_sb = const.tile([D, T], F32)
            tri_sb = const.tile([P, P], MMDT)

            # ---- DRAM scratch for collectives ----
            a2a1_in = dram.tile([8, P, TQ], MMDT)
            a2a1_out = dram.tile([8, P, TQ], MMDT)
            a2a2_in = dram.tile([8, P, TQ], MMDT)
            a2a2_out = dram.tile([8, P, TQ], MMDT)

            # ---- persistent SBUF across phase A (q/k/v resident) ----
            # instance index: inst = 2*m + b  (m: head-in-pair, b: batch)
            qkvp_ctx = tc.tile_pool(name="qkvp", bufs=1)
            qkvp = qkvp_ctx.__enter__()
            qT_sb = qkvp.tile([P, 2 * HPC, T], MMDT, tag="qT_sb", bufs=1)
            kT_sb = qkvp.tile([P, 2 * HPC, T], MMDT, tag="kT_sb", bufs=1)
            v_sb = qkvp.tile([P, NB * T // P, HPC * D], MMDT, tag="v_sb", bufs=1)

            # ================= Phase A1+A2: rmsnorm1 + QKV (chunked) ============
            with (
                tc.tile_pool(name="p12", bufs=2) as p12,
                tc.tile_pool(name="p12psum", bufs=2, space="PSUM") as pp12,
            ):
                CHUNKS = [256, 256, 512, 512, 512]  # per batch (sums to T)
                first_xt = p12.tile([P, CT, CHUNKS[0]], MMDT, tag="xt256", bufs=2)
                nc.sync.dma_start(
                    out=first_xt,
                    in_=x_t[:, 0 : CHUNKS[0]].rearrange("(ct p) t -> p ct t", p=P),
                )
                wq_sb = p12.tile([P, HPC, CT, P], MMDT, tag="wq_sb", bufs=1)
                wk_sb = p12.tile([P, HPC, CT, P], MMDT, tag="wk_sb", bufs=1)
                for m_ in range(HPC):
                    nc.sync.dma_start(out=wq_sb[:, m_], in_=wq[m_])
                for m_ in range(HPC):
                    nc.sync.dma_start(out=wk_sb[:, m_], in_=wk[m_])
                wv_sb = p12.tile([P, CT, HPC * D], MMDT, tag="wv_sb", bufs=1)
                nc.sync.dma_start(out=wv_sb, in_=wv[:, :, :])
                nc.sync.dma_start(out=rope_sb, in_=rope_t[:, :])
                nc.sync.dma_start(out=tri_sb, in_=tri[:, :])

                for b in range(NB):
                    t0p = 0  # within-batch position
                    for ch, CHW in enumerate(CHUNKS):
                        t0 = b * T + t0p  # global column in x_t / v_sb
                        if b == 0 and ch == 0:
                            xt = first_xt
                        else:
                            xt = p12.tile(
                                [P, CT, CHW], MMDT, tag=f"xt{CHW}", bufs=2, name="xt"
                            )
                            nc.sync.dma_start(
                                out=xt,
                                in_=x_t[:, t0 : t0 + CHW].rearrange(
                                    "(ct p) t -> p ct t", p=P
                                ),
                            )
                        # rmsnorm stats: squares on ScalarE, partition-sum on PE
                        sq = p12.tile([P, CT, CHW], MMDT, tag=f"sq{CHW}", bufs=1, name="sq")
                        nc.scalar.activation(
                            sq.rearrange("p a b -> p (a b)"),
                            xt.rearrange("p a b -> p (a b)"),
                            AF.Square,
                        )
                        ssum = pp12.tile([1, TQ], F32, tag="ssum", bufs=1, name="ssum")[:, :CHW]
                        for ct in range(CT):
                            nc.tensor.matmul(
                                ssum,
                                ones_r,
                                sq[:, ct, :],
                                start=(ct == 0),
                                stop=(ct == CT - 1),
                            )
                        srow = p12.tile([1, TQ], F32, tag="srow", bufs=2, name="srow")[:, :CHW]
                        nc.scalar.activation(
                            srow, ssum, AF.Sqrt, bias=eps_sb[0:1, :], scale=1.0 / C
                        )
                        rstd_row = p12.tile([1, TQ], F32, tag="rstd_row", bufs=2, name="rstd_row")[:, :CHW]
                        nc.vector.reciprocal_approx_fast(out=rstd_row, in_=srow)
                        rstd_bc = p12.tile([P, TQ], F32, tag="rstd_bc", bufs=2, name="rstd_bc")[:, :CHW]
                        nc.gpsimd.partition_broadcast(rstd_bc[:], rstd_row[:])
                        # rstd folded into rope tables: rows 0:64 cos*rstd,
                        # 64:128 sin*rstd (q/k matmuls run on RAW x so the PE
                        # never waits on the stats chain)
                        cs_r = p12.tile([P, TQ], F32, tag="cs_r", bufs=2, name="cs_r")[:, :CHW]
                        nc.vector.tensor_tensor(
                            out=cs_r, in0=rope_sb[:, t0p : t0p + CHW], in1=rstd_bc,
                            op=ALU.mult,
                        )

                        # q^T / k^T with fused rope(+rstd) on eviction
                        for w_sb, dst in ((wq_sb, qT_sb), (wk_sb, kT_sb)):
                            for m in range(HPC):
                                inst = 2 * m + b
                                pq = pp12.tile([P, TQ], F32, tag="qk", bufs=3, name="pq")[:, :CHW]
                                for ct in range(CT):
                                    nc.tensor.matmul(
                                        pq,
                                        w_sb[:, m, ct, :],
                                        xt[:, ct, :],
                                        start=(ct == 0),
                                        stop=(ct == CT - 1),
                                    )
                                x1 = pq[0:HD2, :]
                                x2 = pq[HD2:P, :]
                                cosw = cs_r[0:HD2, :]
                                sinw = cs_r[HD2:P, :]
                                tm1 = p12.tile([HD2, TQ], F32, tag="tm1", bufs=2, name="tm1")[:, :CHW]
                                tm2 = p12.tile([HD2, TQ], F32, tag="tm2", bufs=2, name="tm2")[:, :CHW]
                                nc.vector.tensor_tensor(out=tm1, in0=x1, in1=cosw, op=ALU.mult)
                                nc.vector.tensor_tensor(out=tm2, in0=x2, in1=sinw, op=ALU.mult)
                                nc.vector.tensor_tensor(
                                    out=dst[0:HD2, inst, t0p : t0p + CHW],
                                    in0=tm1,
                                    in1=tm2,
                                    op=ALU.subtract,
                                )
                                nc.vector.tensor_tensor(out=tm1, in0=x1, in1=sinw, op=ALU.mult)
                                nc.vector.tensor_tensor(out=tm2, in0=x2, in1=cosw, op=ALU.mult)
                                nc.vector.tensor_tensor(
                                    out=dst[HD2:P, inst, t0p : t0p + CHW],
                                    in0=tm1,
                                    in1=tm2,
                                    op=ALU.add,
                                )

                        # v in row layout [t, 2*D]; per-row rstd via PE-transposed col
                        for rt in range(CHW // P):
                            trp = pp12.tile([P, P], F32, tag="trp", bufs=2)
                            nc.tensor.transpose(
                                trp, rstd_bc[:, rt * P : (rt + 1) * P], ident_f
                            )
                            rstd_col = p12.tile([P, 1], F32, tag="rstd_col", bufs=2)
                            nc.vector.tensor_copy(out=rstd_col, in_=trp[:, 0:1])
                            pv = pp12.tile([P, HPC * D], F32, tag="v", bufs=2)
                            for ct in range(CT):
                                nc.tensor.matmul(
                                    pv,
                                    xt[:, ct, rt * P : (rt + 1) * P],
                                    wv_sb[:, ct, :],
                                    start=(ct == 0),
                                    stop=(ct == CT - 1),
                                )
                            nc.vector.tensor_scalar(
                                out=v_sb[:, t0 // P + rt, :],
                                in0=pv,
                                scalar1=rstd_col,
                                scalar2=None,
                                op0=ALU.mult,
                            )
                        t0p += CHW

            # ================= Phase A3: causal attention (+ A2As) ==============
            with (
                tc.tile_pool(name="att", bufs=2) as att,
                tc.tile_pool(name="attpsum", bufs=2, space="PSUM") as pat,
            ):
                for m in range(HPC):
                    a2a_in = (a2a1_in, a2a2_in)[m]
                    for b in range(NB):
                        inst = 2 * m + b
                        for q2 in range(T // TQA):
                            qb = q2 * TQA
                            o_ps = pat.tile([P, TQA], F32, tag="o", bufs=1)
                            es = []
                            # full key blocks
                            for kb in range(8 * q2):
                                st = pat.tile([P, TQA], F32, tag="st", bufs=2)
                                for i in range(2):
                                    nc.tensor.matmul(
                                        st[:, i * TQ : (i + 1) * TQ],
                                        kT_sb[:, inst, kb * P : (kb + 1) * P],
                                        qT_sb[:, inst, qb + i * TQ : qb + (i + 1) * TQ],
                                        start=True,
                                        stop=True,
                                    )
                                e = att.tile([P, TQA], MMDT, tag="e", bufs=18)
                                nc.scalar.activation(e, st, AF.Exp, scale=inv_sqrt_d)
                                if not es:
                                    eacc = att.tile([P, TQA], MMDT, tag="eacc", bufs=2)
                                    nc.vector.tensor_copy(out=eacc, in_=e)
                                else:
                                    nc.vector.tensor_tensor(
                                        out=eacc, in0=eacc, in1=e, op=ALU.add
                                    )
                                es.append((kb, 0, e))
                            # diagonal blocks (r = 0..7), masked region trimmed
                            for r in range(8):
                                kb = 8 * q2 + r
                                q0 = r * P
                                st = pat.tile([P, TQA], F32, tag="st", bufs=2)
                                if q0 < TQ:
                                    nc.tensor.matmul(
                                        st[:, q0:TQ],
                                        kT_sb[:, inst, kb * P : (kb + 1) * P],
                                        qT_sb[:, inst, qb + q0 : qb + TQ],
                                        start=True,
                                        stop=True,
                                    )
                                    nc.tensor.matmul(
                                        st[:, TQ:TQA],
                                        kT_sb[:, inst, kb * P : (kb + 1) * P],
                                        qT_sb[:, inst, qb + TQ : qb + TQA],
                                        start=True,
                                        stop=True,
                                    )
                                else:
                                    nc.tensor.matmul(
                                        st[:, q0:TQA],
                                        kT_sb[:, inst, kb * P : (kb + 1) * P],
                                        qT_sb[:, inst, qb + q0 : qb + TQA],
                                        start=True,
                                        stop=True,
                                    )
                                e = att.tile([P, TQA], MMDT, tag="e", bufs=18)
                                nc.scalar.activation(
                                    e[:, q0:TQA], st[:, q0:TQA], AF.Exp, scale=inv_sqrt_d
                                )
                                nc.vector.tensor_tensor(
                                    out=e[:, q0 : q0 + P],
                                    in0=e[:, q0 : q0 + P],
                                    in1=tri_sb,
                                    op=ALU.mult,
                                )
                                if not es:
                                    eacc = att.tile([P, TQA], MMDT, tag="eacc", bufs=2)
                                    nc.vector.tensor_copy(out=eacc, in_=e)
                                else:
                                    nc.vector.tensor_tensor(
                                        out=eacc[:, q0:TQA],
                                        in0=eacc[:, q0:TQA],
                                        in1=e[:, q0:TQA],
                                        op=ALU.add,
                                    )
                                es.append((kb, q0, e))
                            n_items = len(es)
                            # last writer of bank0 (cols 0:TQ) is the r=3 diag item;
                            # last writer of bank1 is the final (r=7) item
                            b0_last = n_items - 5
                            # AV pass (the 2 denominator matmuls are emitted
                            # after AV item 1 so their recip/broadcast chain
                            # overlaps the rest of the AV pass)
                            for idx, (kb, q0, e) in enumerate(es):
                                if idx == 2:
                                    l_ps = pat.tile([1, TQA], F32, tag="l", bufs=1)
                                    nc.tensor.matmul(
                                        l_ps[:, 0:TQ], ones_r, eacc[:, 0:TQ],
                                        start=True, stop=True,
                                    )
                                    nc.tensor.matmul(
                                        l_ps[:, TQ:TQA], ones_r, eacc[:, TQ:TQA],
                                        start=True, stop=True,
                                    )
                                first = idx == 0
                                if q0 < TQ:
                                    nc.tensor.matmul(
                                        o_ps[:, q0:TQ],
                                        v_sb[:, b * (T // P) + kb, m * D : (m + 1) * D],
                                        e[:, q0:TQ],
                                        start=first, stop=(idx == b0_last),
                                    )
                                    nc.tensor.matmul(
                                        o_ps[:, TQ:TQA],
                                        v_sb[:, b * (T // P) + kb, m * D : (m + 1) * D],
                                        e[:, TQ:TQA],
                                        start=first, stop=(idx == n_items - 1),
                                    )
                                else:
                                    nc.tensor.matmul(
                                        o_ps[:, q0:TQA],
                                        v_sb[:, b * (T // P) + kb, m * D : (m + 1) * D],
                                        e[:, q0:TQA],
                                        start=first, stop=(idx == n_items - 1),
                                    )
                            l_inv = att.tile([1, TQA], F32, tag="l_inv", bufs=2)
                            nc.vector.reciprocal_approx_fast(out=l_inv, in_=l_ps)
                            l_bc = att.tile([P, TQA], F32, tag="l_bc", bufs=2)
                            nc.gpsimd.partition_broadcast(l_bc[:], l_inv[:])
                            oT = att.tile([P, TQA], MMDT, tag="oT", bufs=2)
                            nc.vector.tensor_tensor(out=oT, in0=o_ps, in1=l_bc, op=ALU.mult)
                            for i in range(2):
                                qc = 2 * q2 + i  # phase-B token block 0..3
                                nc.sync.dma_start(
                                    out=a2a_in[b * 4 + qc, :, :],
                                    in_=oT[:, i * TQ : (i + 1) * TQ],
                                )
                    if m == 0:
                        nc.gpsimd.collective_compute(
                            "AllToAll",
                            ALU.bypass,
                            replica_groups=GROUPS,
                            ins=[a2a1_in.opt()],
                            outs=[a2a1_out.opt()],
                        )
            qkvp_ctx.__exit__(None, None, None)

            # A2A2 emitted outside the attention pool so its completion doesn't
            # gate the pool-close barrier; it overlaps proj pass 0.
            nc.gpsimd.collective_compute(
                "AllToAll",
                ALU.bypass,
                replica_groups=GROUPS,
                ins=[a2a2_in.opt()],
                outs=[a2a2_out.opt()],
            )

            # ---- persistent SBUF through phase B ----
            bper_ctx = tc.tile_pool(name="bper", bufs=1)
            bper = bper_ctx.__enter__()
            xmidT = bper.tile([P, CT, TQ], F32, tag="xmidT", bufs=1)
            h2T = bper.tile([P, CT, TQ], MMDT, tag="h2T", bufs=1)

            # ========== Phase B1: proj^T + residual + rmsnorm2 (transposed) ======
            with (
                tc.tile_pool(name="proj", bufs=2) as prj,
                tc.tile_pool(name="projpsum", bufs=2, space="PSUM") as ppj,
            ):
                lp0 = prj.tile([P, 8, TQ], MMDT, tag="lp0", bufs=1)
                lp1 = prj.tile([P, 8, TQ], MMDT, tag="lp1", bufs=1)
                wpe_first = prj.tile([P, 8, P], MMDT, tag="wpe_first", bufs=1)
                nc.sync.dma_start(out=wpe_first, in_=wpe_r[0, 0, :, 0 : 8 * P])
                # lp0[p, s, t] = a2a1_out[s, p, t]: slot s = head 2s, one DMA.
                nc.sync.dma_start(
                    out=lp0, in_=a2a1_out.rearrange("s p t -> p s t")
                )
                # pass 0: even heads (a2a1), into xmidT
                for ct in range(CT):
                    if ct == 0:
                        wpe_sb = wpe_first
                    else:
                        wpe_sb = prj.tile([P, 8, P], MMDT, tag="wpe_sb", bufs=3)
                        nc.sync.dma_start(out=wpe_sb, in_=wpe_r[0, ct, :, 0 : 8 * P])
                    yps = ppj.tile([P, TQ], F32, tag="y", bufs=4)
                    for blk in range(8):
                        nc.tensor.matmul(
                            yps,
                            wpe_sb[:, blk, :],
                            lp0[:, blk, :],
                            start=(blk == 0),
                            stop=(blk == 7),
                        )
                    nc.scalar.copy(out=xmidT[:, ct, :], in_=yps)
                # residual slice of x (bf16, per-core input)
                xres = prj.tile([P, CT, TQ], MMDT, tag="xres", bufs=1)
                nc.sync.dma_start(
                    out=xres,
                    in_=x_res[:, :].rearrange("(ct p) t -> p ct t", p=P),
                )
                # pass 1: odd heads (a2a2) + residual, rmsnorm2 stats per ct
                nc.sync.dma_start(
                    out=lp1, in_=a2a2_out.rearrange("s p t -> p s t")
                )
                sq2 = bper.tile([P, CT, TQ], MMDT, tag="sq2", bufs=1)
                ssum2 = ppj.tile([1, TQ], F32, tag="ssum2", bufs=1)
                for ct in range(CT):
                    wpe_sb1 = prj.tile([P, 8, P], MMDT, tag="wpe_sb1", bufs=3)
                    nc.sync.dma_start(
                        out=wpe_sb1, in_=wpe_r[1, ct, :, 0 : 8 * P]
                    )
                    yps = ppj.tile([P, TQ], F32, tag="y", bufs=4)
                    for blk in range(8):
                        nc.tensor.matmul(
                            yps,
                            wpe_sb1[:, blk, :],
                            lp1[:, blk, :],
                            start=(blk == 0),
                            stop=(blk == 7),
                        )
                    t1 = prj.tile([P, TQ], F32, tag="t1", bufs=3)
                    nc.vector.tensor_tensor(
                        out=t1, in0=yps, in1=xmidT[:, ct, :], op=ALU.add
                    )
                    nc.vector.tensor_tensor(
                        out=xmidT[:, ct, :], in0=t1, in1=xres[:, ct, :], op=ALU.add
                    )
                    nc.scalar.activation(
                        sq2[:, ct, :], xmidT[:, ct, :], AF.Square
                    )
                    nc.tensor.matmul(
                        ssum2, ones_r, sq2[:, ct, :], start=(ct == 0), stop=(ct == CT - 1)
                    )
                srow2 = bper.tile([1, TQ], F32, tag="srow2", bufs=1)
                nc.scalar.activation(
                    srow2, ssum2, AF.Sqrt, bias=eps_sb[0:1, :], scale=1.0 / C
                )
                rstd2 = bper.tile([1, TQ], F32, tag="rstd2", bufs=1)
                nc.vector.reciprocal_approx_fast(out=rstd2, in_=srow2)
                rstd2_bc = bper.tile([P, TQ], F32, tag="rstd2_bc", bufs=1)
                nc.gpsimd.partition_broadcast(rstd2_bc[:], rstd2[:])
                for ct in range(CT):
                    nc.vector.tensor_tensor(
                        out=h2T[:, ct, :], in0=xmidT[:, ct, :], in1=rstd2_bc, op=ALU.mult
                    )

            # ================= Phase B2: SwiGLU (transposed w3 pass) =============
            with (
                tc.tile_pool(name="mlp", bufs=2) as mlp,
                tc.tile_pool(name="mlppsum", bufs=2, space="PSUM") as pml,
            ):
                uT = mlp.tile([P, HID_T, TQ], MMDT, tag="uT", bufs=1)
                for ht in range(HID_T):
                    w1_sb = mlp.tile([P, CT, P], MMDT, tag="w1_sb", bufs=3)
                    nc.sync.dma_start(out=w1_sb, in_=w1t[ht])
                    w2_sb = mlp.tile([P, CT, P], MMDT, tag="w2_sb", bufs=3)
                    nc.sync.dma_start(out=w2_sb, in_=w2t[ht])
                    g1 = pml.tile([P, TQ], F32, tag="g1", bufs=2)
                    g2 = pml.tile([P, TQ], F32, tag="g2", bufs=2)
                    for ct in range(CT):
                        nc.tensor.matmul(
                            g1, w1_sb[:, ct, :], h2T[:, ct, :],
                            start=(ct == 0), stop=(ct == CT - 1),
                        )
                    for ct in range(CT):
                        nc.tensor.matmul(
                            g2, w2_sb[:, ct, :], h2T[:, ct, :],
                            start=(ct == 0), stop=(ct == CT - 1),
                        )
                    sil = mlp.tile([P, TQ], F32, tag="sil", bufs=3)
                    nc.scalar.activation(sil, g1, AF.Silu)
                    nc.vector.tensor_tensor(
                        out=uT[:, ht, :], in0=g2, in1=sil, op=ALU.mult
                    )
                # y3^T: stationary w3 blocks, moving uT; accumulate 44 ht per ct
                for ct in range(CT):
                    w3_sb = mlp.tile([P, HID_T, P], MMDT, tag="w3_sb", bufs=2)
                    nc.sync.dma_start(out=w3_sb, in_=w3r[ct])
                    y3 = pml.tile([P, TQ], F32, tag="y3", bufs=2)
                    for ht in range(HID_T):
                        nc.tensor.matmul(
                            y3, w3_sb[:, ht, :], uT[:, ht, :],
                            start=(ht == 0), stop=(ht == HID_T - 1),
                        )
                    ofin = mlp.tile([P, TQ], F32, tag="ofin", bufs=3)
                    nc.vector.tensor_tensor(
                        out=ofin, in0=y3, in1=xmidT[:, ct, :], op=ALU.add
                    )
                    nc.sync.dma_start(out=out[ct * P : (ct + 1) * P, :], in_=ofin)
            bper_ctx.__exit__(None, None, None)

    nc.compile()
    return nc


_NC_CACHE = None


def _get_nc():
    global _NC_CACHE
    if _NC_CACHE is None:
        _NC_CACHE = _build()
    return _NC_CACHE


def _host_inputs(x, w_norm1, w_qkv, w_proj, w_norm2, w1, w2, w3):
    x = np.asarray(x, dtype=np.float32)
    w_qkv = np.asarray(w_qkv, dtype=np.float32)
    w_proj = np.asarray(w_proj, dtype=np.float32)
    w_norm1 = np.asarray(w_norm1, dtype=np.float32)
    w_norm2 = np.asarray(w_norm2, dtype=np.float32)
    w1 = np.asarray(w1, dtype=np.float32)
    w2 = np.asarray(w2, dtype=np.float32)
    w3 = np.asarray(w3, dtype=np.float32)

    half = D // 2
    inv_freq = 1.0 / (ROPE_BASE ** (np.arange(half, dtype=np.float32) / half))
    pos = np.arange(T, dtype=np.float32)
    freqs = pos[:, None] * inv_freq[None, :]
    rope_tab = np.ascontiguousarray(
        np.concatenate([np.cos(freqs).T, np.sin(freqs).T], axis=0).astype(np.float32)
    )

    ql = np.arange(P)[None, :]
    kv = np.arange(P)[:, None]
    tri = (ql >= kv).astype(NP_MMDT)

    # fold w_norm into weight rows (h @ W == (x*rstd) @ (diag(wn) W))
    w_qkv_n = w_qkv * w_norm1[:, None]
    w1_n = w1 * w_norm2[:, None]
    w2_n = w2 * w_norm2[:, None]

    # [HID_T, P, CT*P]: w1t[ht, p, ct*P + d] = w1_n[ct*P + p, ht*P + d]
    w1t = np.ascontiguousarray(
        w1_n.reshape(CT, P, HID_T, P).transpose(2, 1, 0, 3).reshape(HID_T, P, C)
    ).astype(NP_MMDT)
    w2t = np.ascontiguousarray(
        w2_n.reshape(CT, P, HID_T, P).transpose(2, 1, 0, 3).reshape(HID_T, P, C)
    ).astype(NP_MMDT)
    # [CT, P, HID_T*P]: w3r[ct, p, ht*P + d] = w3[ht*P + p, ct*P + d]
    w3r_h = np.ascontiguousarray(
        w3.reshape(HID_T, P, CT, P).transpose(2, 1, 0, 3).reshape(CT, P, HID)
    ).astype(NP_MMDT)

    # [P, CT, cols]: wqkv_r[p, ct, d] = w_qkv_n[ct*P + p, d]
    wqkv_r = np.ascontiguousarray(
        w_qkv_n.reshape(CT, P, 3 * C).transpose(1, 0, 2)
    ).astype(NP_MMDT)

    # wpe: [2, CT, P, 8*P].  Pass p block s = w_proj rows of head (2s + p)
    # (receiver slot s carries head 2s for pass 0, head 2s+1 for pass 1).
    wpe_full = np.empty((2, 8, P, C), dtype=np.float32)
    for s_ in range(8):
        wpe_full[0, s_] = w_proj[(2 * s_) * P : (2 * s_ + 1) * P, :]
        wpe_full[1, s_] = w_proj[(2 * s_ + 1) * P : (2 * s_ + 2) * P, :]
    wpe_r_h = np.ascontiguousarray(
        wpe_full.reshape(2, 8, P, CT, P).transpose(0, 3, 2, 1, 4).reshape(2, CT, P, 8 * P)
    ).astype(NP_MMDT)

    # x_t: [C, 2T] both batches, bf16 (shared across cores)
    x_t_h = np.ascontiguousarray(
        np.concatenate([x[0].T, x[1].T], axis=1)
    ).astype(NP_MMDT)

    in_maps = []
    for j in range(8):
        b, hg = j // 4, j % 4
        col0 = 2 * j * D  # first head of this core's pair
        in_maps.append(
            {
                "x_t": x_t_h,
                "wq": np.ascontiguousarray(
                    wqkv_r[:, :, col0 : col0 + HPC * D]
                    .reshape(P, CT, HPC, P)
                    .transpose(2, 0, 1, 3)
                    .reshape(HPC, P, C)
                ),
                "wk": np.ascontiguousarray(
                    wqkv_r[:, :, C + col0 : C + col0 + HPC * D]
                    .reshape(P, CT, HPC, P)
                    .transpose(2, 0, 1, 3)
                    .reshape(HPC, P, C)
                ),
                "wv": np.ascontiguousarray(
                    wqkv_r[:, :, 2 * C + col0 : 2 * C + col0 + HPC * D]
                ),
                "wpe_r": wpe_r_h,
                "w1t": w1t,
                "w2t": w2t,
                "w3r": w3r_h,
                "rope_t": rope_tab,
                "tri": tri,
                "x_res": np.ascontiguousarray(
                    x_t_h[:, b * T + hg * TQ : b * T + (hg + 1) * TQ]
                ),
            }
        )
    return in_maps


def kernel(x, w_norm1, w_qkv, w_proj, w_norm2, w1, w2, w3, _trace=False, _tmpdir=None):
    nc = _get_nc()
    in_maps = _host_inputs(x, w_norm1, w_qkv, w_proj, w_norm2, w1, w2, w3)
    kwargs = {}
    if _trace:
        kwargs = {"trace": True, "tmpdir": _tmpdir}
    res = bass_utils.run_bass_kernel_spmd(
        nc, in_maps, core_ids=list(range(8)), **kwargs
    )
    out = np.empty((2, T, C), dtype=np.float32)
    for j in range(8):
        out[j // 4, (j % 4) * TQ : (j % 4 + 1) * TQ, :] = res.results[j]["out"].T
    kernel._last_exec_time_ns = res.exec_time_ns
    return out


# revision 22
# speedup vs baseline: 1.1160x; 1.0407x over previous
"""Dense transformer block (rmsnorm+causal attention+rope / rmsnorm+SwiGLU) on 8 TRN2 cores.

Sharding v2:
  core j: heads {2j, 2j+1} x BOTH batches (4 head-instances).  Every core
  therefore holds attention output destined for all 8 phase-B owners, so the
  two AllToAlls carry fully-dense payload (no cross-batch zero slots, no
  bmask, no receiver slot-pair sums).  A2A1 fires after the even-head
  instances (50% of attention), A2A2 after the odd-head instances; proj
  pass 0 overlaps A2A2's flight.
  Phase B owner of core j: batch j//4, token block j%4 (as baseline).

  Phase A: rmsnorm1 stats via ScalarE squares + DVE ct-accumulation + one
  ones-matmul per chunk; q/k/v matmuls run on RAW x (rstd folded into the
  rope tables for q/k, transposed per-row scale for v) so the PE never
  waits on the stats chain.  Causal attention keeps q/k/v SBUF-resident;
  softmax denominators are DVE-accumulated (bf16 eacc, interleaved with
  exp) + two 512-col ones-matmuls emitted after the AV pass.
  Phase B runs fully transposed: proj y^T accumulates [C, TQ], residual
  read straight from x_t (bf16), rmsnorm2 + SwiGLU with a transposed w3
  pass; output is [C, TQ] per core, transposed on host.

Matmul operands are bf16 (weights and x pre-cast on host, w_norm folded into
weight rows); statistics, softmax sums, residual accumulation and PSUM stay
fp32.
"""

import numpy as np
import ml_dtypes

import concourse.bass as bass
import concourse.mybir as mybir
import concourse.tile as tile
from concourse import bacc
from concourse import bass_utils
from concourse.masks import make_identity

AF = mybir.ActivationFunctionType
ALU = mybir.AluOpType
F32 = mybir.dt.float32
BF16 = mybir.dt.bfloat16
MMDT = BF16
NP_MMDT = ml_dtypes.bfloat16

P = 128
T = 2048
C = 2048
D = 128
H = 16
HPC = 2          # heads per core
NB = 2           # batches (all processed on every core)
HID = 5632
HID_T = HID // P  # 44 hid tiles
TQ = 512         # A2A / output col-block granularity
TQA = 1024       # attention query-chunk
EPS = 1e-6
ROPE_BASE = 10000.0
CT = C // P      # 16 contraction tiles
HD2 = D // 2


def _build():
    nc = bacc.Bacc(None, target_bir_lowering=False, num_devices=8)

    # ---- kernel I/O ----
    x_t = nc.dram_tensor("x_t", [C, NB * T], MMDT, kind="ExternalInput")
    wq = nc.dram_tensor("wq", [HPC, P, CT * P], MMDT, kind="ExternalInput")
    wk = nc.dram_tensor("wk", [HPC, P, CT * P], MMDT, kind="ExternalInput")
    wv = nc.dram_tensor("wv", [P, CT, HPC * D], MMDT, kind="ExternalInput")
    wpe_r = nc.dram_tensor("wpe_r", [2, CT, P, 8 * P], MMDT, kind="ExternalInput")
    w1t = nc.dram_tensor("w1t", [HID_T, P, CT * P], MMDT, kind="ExternalInput")
    w2t = nc.dram_tensor("w2t", [HID_T, P, CT * P], MMDT, kind="ExternalInput")
    w3r = nc.dram_tensor("w3r", [CT, P, HID_T * P], MMDT, kind="ExternalInput")
    rope_t = nc.dram_tensor("rope_t", [D, T], F32, kind="ExternalInput")
    tri = nc.dram_tensor("tri", [P, P], MMDT, kind="ExternalInput")
    x_res = nc.dram_tensor("x_res", [C, TQ], MMDT, kind="ExternalInput")
    out = nc.dram_tensor("out", [C, TQ], F32, kind="ExternalOutput")

    inv_sqrt_d = 1.0 / float(np.sqrt(D))
    GROUPS = [[0, 1, 2, 3, 4, 5, 6, 7]]

    with tile.TileContext(nc) as tc:
        with (
            tc.tile_pool(name="const", bufs=1) as const,
            tc.tile_pool(name="dram", bufs=1, space="DRAM") as dram,
        ):
            # ---- constants ----
            ones_f = const.tile([P, 1], F32)
            nc.vector.memset(ones_f, 1.0)
            ones_r = const.tile([P, 1], MMDT)
            nc.vector.tensor_copy(out=ones_r, in_=ones_f)
            eps_sb = const.tile([P, 1], F32)
            nc.vector.memset(eps_sb, EPS)
            ident_f = const.tile([P, P], F32)
            make_identity(nc, ident_f)
            rope_sb = const.tile([D, T], F32)
            tri_sb = const.tile([P, P], MMDT)

            # ---- DRAM scratch for collectives ----
            a2a1_in = dram.tile([8, P, TQ], MMDT)
            a2a1_out = dram.tile([8, P, TQ], MMDT)
            a2a2_in = dram.tile([8, P, TQ], MMDT)
            a2a2_out = dram.tile([8, P, TQ], MMDT)

            # ---- persistent SBUF across phase A (q/k/v resident) ----
            # instance index: inst = 2*m + b  (m: head-in-pair, b: batch)
            qkvp_ctx = tc.tile_pool(name="qkvp", bufs=1)
            qkvp = qkvp_ctx.__enter__()
            qT_sb = qkvp.tile([P, 2 * HPC, T], MMDT, tag="qT_sb", bufs=1)
            kT_sb = qkvp.tile([P, 2 * HPC, T], MMDT, tag="kT_sb", bufs=1)
            v_sb = qkvp.tile([P, NB * T // P, HPC * D], MMDT, tag="v_sb", bufs=1)

            # ================= Phase A1+A2: rmsnorm1 + QKV (chunked) ============
            with (
                tc.tile_pool(name="p12", bufs=2) as p12,
                tc.tile_pool(name="p12psum", bufs=2, space="PSUM") as pp12,
            ):
                CHUNKS = [256, 256, 512, 512, 512]  # per batch (sums to T)
                wq_sb = p12.tile([P, HPC, CT, P], MMDT, tag="wq_sb", bufs=1)
                wk_sb = p12.tile([P, HPC, CT, P], MMDT, tag="wk_sb", bufs=1)
                nc.sync.dma_start(out=wq_sb[:, 0], in_=wq[0])
                first_xt = p12.tile([P, CT, CHUNKS[0]], MMDT, tag="xt256", bufs=2)
                nc.sync.dma_start(
                    out=first_xt,
                    in_=x_t[:, 0 : CHUNKS[0]].rearrange("(ct p) t -> p ct t", p=P),
                )
                for m_ in range(1, HPC):
                    nc.sync.dma_start(out=wq_sb[:, m_], in_=wq[m_])
                for m_ in range(HPC):
                    nc.sync.dma_start(out=wk_sb[:, m_], in_=wk[m_])
                wv_sb = p12.tile([P, CT, HPC * D], MMDT, tag="wv_sb", bufs=1)
                nc.sync.dma_start(out=wv_sb, in_=wv[:, :, :])
                nc.sync.dma_start(out=rope_sb, in_=rope_t[:, :])
                nc.sync.dma_start(out=tri_sb, in_=tri[:, :])

                for b in range(NB):
                    t0p = 0  # within-batch position
                    for ch, CHW in enumerate(CHUNKS):
                        t0 = b * T + t0p  # global column in x_t / v_sb
                        if b == 0 and ch == 0:
                            xt = first_xt
                        else:
                            xt = p12.tile(
                                [P, CT, CHW], MMDT, tag=f"xt{CHW}", bufs=2, name="xt"
                            )
                            nc.sync.dma_start(
                                out=xt,
                                in_=x_t[:, t0 : t0 + CHW].rearrange(
                                    "(ct p) t -> p ct t", p=P
                                ),
                            )
                        # rmsnorm stats: squares on ScalarE, partition-sum on PE
                        sq = p12.tile([P, CT, CHW], MMDT, tag=f"sq{CHW}", bufs=1, name="sq")
                        nc.scalar.activation(
                            sq.rearrange("p a b -> p (a b)"),
                            xt.rearrange("p a b -> p (a b)"),
                            AF.Square,
                        )
                        ssum = pp12.tile([1, TQ], F32, tag="ssum", bufs=1, name="ssum")[:, :CHW]
                        for ct in range(CT):
                            nc.tensor.matmul(
                                ssum,
                                ones_r,
                                sq[:, ct, :],
                                start=(ct == 0),
                                stop=(ct == CT - 1),
                            )
                        srow = p12.tile([1, TQ], F32, tag="srow", bufs=2, name="srow")[:, :CHW]
                        nc.scalar.activation(
                            srow, ssum, AF.Sqrt, bias=eps_sb[0:1, :], scale=1.0 / C
                        )
                        rstd_row = p12.tile([1, TQ], F32, tag="rstd_row", bufs=2, name="rstd_row")[:, :CHW]
                        nc.vector.reciprocal_approx_fast(out=rstd_row, in_=srow)
                        rstd_bc = p12.tile([P, TQ], F32, tag="rstd_bc", bufs=2, name="rstd_bc")[:, :CHW]
                        nc.gpsimd.partition_broadcast(rstd_bc[:], rstd_row[:])
                        # rstd folded into rope tables: rows 0:64 cos*rstd,
                        # 64:128 sin*rstd (q/k matmuls run on RAW x so the PE
                        # never waits on the stats chain)
                        cs_r = p12.tile([P, TQ], F32, tag="cs_r", bufs=2, name="cs_r")[:, :CHW]
                        nc.vector.tensor_tensor(
                            out=cs_r, in0=rope_sb[:, t0p : t0p + CHW], in1=rstd_bc,
                            op=ALU.mult,
                        )

                        # q^T / k^T with fused rope(+rstd) on eviction
                        for w_sb, dst in ((wq_sb, qT_sb), (wk_sb, kT_sb)):
                            for m in range(HPC):
                                inst = 2 * m + b
                                pq = pp12.tile([P, TQ], F32, tag="qk", bufs=3, name="pq")[:, :CHW]
                                for ct in range(CT):
                                    nc.tensor.matmul(
                                        pq,
                                        w_sb[:, m, ct, :],
                                        xt[:, ct, :],
                                        start=(ct == 0),
                                        stop=(ct == CT - 1),
                                    )
                                x1 = pq[0:HD2, :]
                                x2 = pq[HD2:P, :]
                                cosw = cs_r[0:HD2, :]
                                sinw = cs_r[HD2:P, :]
                                tm1 = p12.tile([HD2, TQ], F32, tag="tm1", bufs=2, name="tm1")[:, :CHW]
                                tm2 = p12.tile([HD2, TQ], F32, tag="tm2", bufs=2, name="tm2")[:, :CHW]
                                nc.vector.tensor_tensor(out=tm1, in0=x1, in1=cosw, op=ALU.mult)
                                nc.vector.tensor_tensor(out=tm2, in0=x2, in1=sinw, op=ALU.mult)
                                nc.vector.tensor_tensor(
                                    out=dst[0:HD2, inst, t0p : t0p + CHW],
                                    in0=tm1,
                                    in1=tm2,
                                    op=ALU.subtract,
                                )
                                nc.vector.tensor_tensor(out=tm1, in0=x1, in1=sinw, op=ALU.mult)
                                nc.vector.tensor_tensor(out=tm2, in0=x2, in1=cosw, op=ALU.mult)
                                nc.vector.tensor_tensor(
                                    out=dst[HD2:P, inst, t0p : t0p + CHW],
                                    in0=tm1,
                                    in1=tm2,
                                    op=ALU.add,
                                )

                        # v in row layout [t, 2*D]; per-row rstd via PE-transposed col
                        for rt in range(CHW // P):
                            trp = pp12.tile([P, P], F32, tag="trp", bufs=2)
                            nc.tensor.transpose(
                                trp, rstd_bc[:, rt * P : (rt + 1) * P], ident_f
                            )
                            rstd_col = p12.tile([P, 1], F32, tag="rstd_col", bufs=2)
                            nc.vector.tensor_copy(out=rstd_col, in_=trp[:, 0:1])
                            pv = pp12.tile([P, HPC * D], F32, tag="v", bufs=2)
                            for ct in range(CT):
                                nc.tensor.matmul(
                                    pv,
                                    xt[:, ct, rt * P : (rt + 1) * P],
                                    wv_sb[:, ct, :],
                                    start=(ct == 0),
                                    stop=(ct == CT - 1),
                                )
                            nc.vector.tensor_scalar(
                                out=v_sb[:, t0 // P + rt, :],
                                in0=pv,
                                scalar1=rstd_col,
                                scalar2=None,
                                op0=ALU.mult,
                            )
                        t0p += CHW

            # ================= Phase A3: causal attention (+ A2As) ==============
            with (
                tc.tile_pool(name="att", bufs=2) as att,
                tc.tile_pool(name="attpsum", bufs=2, space="PSUM") as pat,
            ):
                for m in range(HPC):
                    a2a_in = (a2a1_in, a2a2_in)[m]
                    for b in range(NB):
                        inst = 2 * m + b
                        for q2 in range(T // TQA):
                            qb = q2 * TQA
                            o_ps = pat.tile([P, TQA], F32, tag="o", bufs=1)
                            es = []
                            n_items = 8 * q2 + 8
                            # last writer of bank0 (cols 0:TQ) is the r=3 diag
                            # item; last writer of bank1 is the final item
                            b0_last = n_items - 5

                            def av_emit(idx):
                                kb, q0, e = es[idx]
                                first = idx == 0
                                if q0 < TQ:
                                    nc.tensor.matmul(
                                        o_ps[:, q0:TQ],
                                        v_sb[:, b * (T // P) + kb, m * D : (m + 1) * D],
                                        e[:, q0:TQ],
                                        start=first, stop=(idx == b0_last),
                                    )
                                    nc.tensor.matmul(
                                        o_ps[:, TQ:TQA],
                                        v_sb[:, b * (T // P) + kb, m * D : (m + 1) * D],
                                        e[:, TQ:TQA],
                                        start=first, stop=(idx == n_items - 1),
                                    )
                                else:
                                    nc.tensor.matmul(
                                        o_ps[:, q0:TQA],
                                        v_sb[:, b * (T // P) + kb, m * D : (m + 1) * D],
                                        e[:, q0:TQA],
                                        start=first, stop=(idx == n_items - 1),
                                    )
                            # full key blocks
                            for kb in range(8 * q2):
                                st = pat.tile([P, TQA], F32, tag="st", bufs=2)
                                for i in range(2):
                                    nc.tensor.matmul(
                                        st[:, i * TQ : (i + 1) * TQ],
                                        kT_sb[:, inst, kb * P : (kb + 1) * P],
                                        qT_sb[:, inst, qb + i * TQ : qb + (i + 1) * TQ],
                                        start=True,
                                        stop=True,
                                    )
                                e = att.tile([P, TQA], MMDT, tag="e", bufs=18)
                                nc.scalar.activation(e, st, AF.Exp, scale=inv_sqrt_d)
                                if not es:
                                    eacc = att.tile([P, TQA], MMDT, tag="eacc", bufs=2)
                                    nc.vector.tensor_copy(out=eacc, in_=e)
                                else:
                                    nc.vector.tensor_tensor(
                                        out=eacc, in0=eacc, in1=e, op=ALU.add
                                    )
                                es.append((kb, 0, e))
                                if len(es) >= 4:
                                    av_emit(len(es) - 4)
                            # diagonal blocks (r = 0..7), masked region trimmed
                            for r in range(8):
                                kb = 8 * q2 + r
                                q0 = r * P
                                st = pat.tile([P, TQA], F32, tag="st", bufs=2)
                                if q0 < TQ:
                                    nc.tensor.matmul(
                                        st[:, q0:TQ],
                                        kT_sb[:, inst, kb * P : (kb + 1) * P],
                                        qT_sb[:, inst, qb + q0 : qb + TQ],
                                        start=True,
                                        stop=True,
                                    )
                                    nc.tensor.matmul(
                                        st[:, TQ:TQA],
                                        kT_sb[:, inst, kb * P : (kb + 1) * P],
                                        qT_sb[:, inst, qb + TQ : qb + TQA],
                                        start=True,
                                        stop=True,
                                    )
                                else:
                                    nc.tensor.matmul(
                                        st[:, q0:TQA],
                                        kT_sb[:, inst, kb * P : (kb + 1) * P],
                                        qT_sb[:, inst, qb + q0 : qb + TQA],
                                        start=True,
                                        stop=True,
                                    )
                                e = att.tile([P, TQA], MMDT, tag="e", bufs=18)
                                nc.scalar.activation(
                                    e[:, q0:TQA], st[:, q0:TQA], AF.Exp, scale=inv_sqrt_d
                                )
                                nc.vector.tensor_tensor(
                                    out=e[:, q0 : q0 + P],
                                    in0=e[:, q0 : q0 + P],
                                    in1=tri_sb,
                                    op=ALU.mult,
                                )
                                if not es:
                                    eacc = att.tile([P, TQA], MMDT, tag="eacc", bufs=2)
                                    nc.vector.tensor_copy(out=eacc, in_=e)
                                else:
                                    nc.vector.tensor_tensor(
                                        out=eacc[:, q0:TQA],
                                        in0=eacc[:, q0:TQA],
                                        in1=e[:, q0:TQA],
                                        op=ALU.add,
                                    )
                                es.append((kb, q0, e))
                                if len(es) >= 4:
                                    av_emit(len(es) - 4)
                            # drain the AV pipeline, then the denominator
                            # matmuls (their recip/bcast/oT chain overlaps the
                            # next group's QK pass)
                            for idx in range(n_items - 3, n_items):
                                av_emit(idx)
                            l_ps = pat.tile([1, TQA], F32, tag="l", bufs=1)
                            nc.tensor.matmul(
                                l_ps[:, 0:TQ], ones_r, eacc[:, 0:TQ],
                                start=True, stop=True,
                            )
                            nc.tensor.matmul(
                                l_ps[:, TQ:TQA], ones_r, eacc[:, TQ:TQA],
                                start=True, stop=True,
                            )
                            l_inv = att.tile([1, TQA], F32, tag="l_inv", bufs=2)
                            nc.vector.reciprocal_approx_fast(out=l_inv, in_=l_ps)
                            l_bc = att.tile([P, TQA], F32, tag="l_bc", bufs=2)
                            nc.gpsimd.partition_broadcast(l_bc[:], l_inv[:])
                            oT = att.tile([P, TQA], MMDT, tag="oT", bufs=2)
                            nc.vector.tensor_tensor(out=oT, in0=o_ps, in1=l_bc, op=ALU.mult)
                            for i in range(2):
                                qc = 2 * q2 + i  # phase-B token block 0..3
                                nc.sync.dma_start(
                                    out=a2a_in[b * 4 + qc, :, :],
                                    in_=oT[:, i * TQ : (i + 1) * TQ],
                                )
                    if m == 0:
                        nc.gpsimd.collective_compute(
                            "AllToAll",
                            ALU.bypass,
                            replica_groups=GROUPS,
                            ins=[a2a1_in.opt()],
                            outs=[a2a1_out.opt()],
                        )
            qkvp_ctx.__exit__(None, None, None)

            # A2A2 emitted outside the attention pool so its completion doesn't
            # gate the pool-close barrier; it overlaps proj pass 0.
            nc.gpsimd.collective_compute(
                "AllToAll",
                ALU.bypass,
                replica_groups=GROUPS,
                ins=[a2a2_in.opt()],
                outs=[a2a2_out.opt()],
            )

            # ---- persistent SBUF through phase B ----
            bper_ctx = tc.tile_pool(name="bper", bufs=1)
            bper = bper_ctx.__enter__()
            xmidT = bper.tile([P, CT, TQ], F32, tag="xmidT", bufs=1)
            h2T = bper.tile([P, CT, TQ], MMDT, tag="h2T", bufs=1)

            # ========== Phase B1: proj^T + residual + rmsnorm2 (transposed) ======
            with (
                tc.tile_pool(name="proj", bufs=2) as prj,
                tc.tile_pool(name="projpsum", bufs=2, space="PSUM") as ppj,
            ):
                lp0 = prj.tile([P, 8, TQ], MMDT, tag="lp0", bufs=1)
                lp1 = prj.tile([P, 8, TQ], MMDT, tag="lp1", bufs=1)
                wpe_first = prj.tile([P, 8, P], MMDT, tag="wpe_first", bufs=1)
                nc.sync.dma_start(out=wpe_first, in_=wpe_r[0, 0, :, 0 : 8 * P])
                # lp0[p, s, t] = a2a1_out[s, p, t]: slot s = head 2s, one DMA.
                nc.sync.dma_start(
                    out=lp0, in_=a2a1_out.rearrange("s p t -> p s t")
                )
                # pass 0: even heads (a2a1), into xmidT
                for ct in range(CT):
                    if ct == 0:
                        wpe_sb = wpe_first
                    else:
                        wpe_sb = prj.tile([P, 8, P], MMDT, tag="wpe_sb", bufs=3)
                        nc.sync.dma_start(out=wpe_sb, in_=wpe_r[0, ct, :, 0 : 8 * P])
                    yps = ppj.tile([P, TQ], F32, tag="y", bufs=4)
                    for blk in range(8):
                        nc.tensor.matmul(
                            yps,
                            wpe_sb[:, blk, :],
                            lp0[:, blk, :],
                            start=(blk == 0),
                            stop=(blk == 7),
                        )
                    nc.scalar.copy(out=xmidT[:, ct, :], in_=yps)
                # residual slice of x (bf16, per-core input)
                xres = prj.tile([P, CT, TQ], MMDT, tag="xres", bufs=1)
                nc.sync.dma_start(
                    out=xres,
                    in_=x_res[:, :].rearrange("(ct p) t -> p ct t", p=P),
                )
                # pass 1: odd heads (a2a2) + residual, rmsnorm2 stats per ct
                nc.sync.dma_start(
                    out=lp1, in_=a2a2_out.rearrange("s p t -> p s t")
                )
                sq2 = bper.tile([P, CT, TQ], MMDT, tag="sq2", bufs=1)
                ssum2 = ppj.tile([1, TQ], F32, tag="ssum2", bufs=1)
                for ct in range(CT):
                    wpe_sb1 = prj.tile([P, 8, P], MMDT, tag="wpe_sb1", bufs=3)
                    nc.sync.dma_start(
                        out=wpe_sb1, in_=wpe_r[1, ct, :, 0 : 8 * P]
                    )
                    yps = ppj.tile([P, TQ], F32, tag="y", bufs=4)
                    for blk in range(8):
                        nc.tensor.matmul(
                            yps,
                            wpe_sb1[:, blk, :],
                            lp1[:, blk, :],
                            start=(blk == 0),
                            stop=(blk == 7),
                        )
                    t1 = prj.tile([P, TQ], F32, tag="t1", bufs=3)
                    nc.vector.tensor_tensor(
                        out=t1, in0=yps, in1=xmidT[:, ct, :], op=ALU.add
                    )
                    nc.vector.tensor_tensor(
                        out=xmidT[:, ct, :], in0=t1, in1=xres[:, ct, :], op=ALU.add
                    )
                    nc.scalar.activation(
                        sq2[:, ct, :], xmidT[:, ct, :], AF.Square
                    )
                    nc.tensor.matmul(
                        ssum2, ones_r, sq2[:, ct, :], start=(ct == 0), stop=(ct == CT - 1)
                    )
                srow2 = bper.tile([1, TQ], F32, tag="srow2", bufs=1)
                nc.scalar.activation(
                    srow2, ssum2, AF.Sqrt, bias=eps_sb[0:1, :], scale=1.0 / C
                )
                rstd2 = bper.tile([1, TQ], F32, tag="rstd2", bufs=1)
                nc.vector.reciprocal_approx_fast(out=rstd2, in_=srow2)
                rstd2_bc = bper.tile([P, TQ], F32, tag="rstd2_bc", bufs=1)
                nc.gpsimd.partition_broadcast(rstd2_bc[:], rstd2[:])
                for ct in range(CT):
                    nc.vector.tensor_tensor(
                        out=h2T[:, ct, :], in0=xmidT[:, ct, :], in1=rstd2_bc, op=ALU.mult
                    )

            # ================= Phase B2: SwiGLU (transposed w3 pass) =============
            with (
                tc.tile_pool(name="mlp", bufs=2) as mlp,
                tc.tile_pool(name="mlppsum", bufs=2, space="PSUM") as pml,
            ):
                uT = mlp.tile([P, HID_T, TQ], MMDT, tag="uT", bufs=1)
                for ht in range(HID_T):
                    w1_sb = mlp.tile([P, CT, P], MMDT, tag="w1_sb", bufs=3)
                    nc.sync.dma_start(out=w1_sb, in_=w1t[ht])
                    w2_sb = mlp.tile([P, CT, P], MMDT, tag="w2_sb", bufs=3)
                    nc.sync.dma_start(out=w2_sb, in_=w2t[ht])
                    g1 = pml.tile([P, TQ], F32, tag="g1", bufs=2)
                    g2 = pml.tile([P, TQ], F32, tag="g2", bufs=2)
                    for ct in range(CT):
                        nc.tensor.matmul(
                            g1, w1_sb[:, ct, :], h2T[:, ct, :],
                            start=(ct == 0), stop=(ct == CT - 1),
                        )
                    for ct in range(CT):
                        nc.tensor.matmul(
                            g2, w2_sb[:, ct, :], h2T[:, ct, :],
                            start=(ct == 0), stop=(ct == CT - 1),
                        )
                    sil = mlp.tile([P, TQ], F32, tag="sil", bufs=3)
                    nc.scalar.activation(sil, g1, AF.Silu)
                    nc.vector.tensor_tensor(
                        out=uT[:, ht, :], in0=g2, in1=sil, op=ALU.mult
                    )
                # y3^T: stationary w3 blocks, moving uT; accumulate 44 ht per ct
                for ct in range(CT):
                    w3_sb = mlp.tile([P, HID_T, P], MMDT, tag="w3_sb", bufs=2)
                    nc.sync.dma_start(out=w3_sb, in_=w3r[ct])
                    y3 = pml.tile([P, TQ], F32, tag="y3", bufs=2)
                    for ht in range(HID_T):
                        nc.tensor.matmul(
                            y3, w3_sb[:, ht, :], uT[:, ht, :],
                            start=(ht == 0), stop=(ht == HID_T - 1),
                        )
                    ofin = mlp.tile([P, TQ], F32, tag="ofin", bufs=3)
                    nc.vector.tensor_tensor(
                        out=ofin, in0=y3, in1=xmidT[:, ct, :], op=ALU.add
                    )
                    nc.sync.dma_start(out=out[ct * P : (ct + 1) * P, :], in_=ofin)
            bper_ctx.__exit__(None, None, None)

    nc.compile()
    return nc


_NC_CACHE = None


def _get_nc():
    global _NC_CACHE
    if _NC_CACHE is None:
        _NC_CACHE = _build()
    return _NC_CACHE


def _host_inputs(x, w_norm1, w_qkv, w_proj, w_norm2, w1, w2, w3):
    x = np.asarray(x, dtype=np.float32)
    w_qkv = np.asarray(w_qkv, dtype=np.float32)
    w_proj = np.asarray(w_proj, dtype=np.float32)
    w_norm1 = np.asarray(w_norm1, dtype=np.float32)
    w_norm2 = np.asarray(w_norm2, dtype=np.float32)
    w1 = np.asarray(w1, dtype=np.float32)
    w2 = np.asarray(w2, dtype=np.float32)
    w3 = np.asarray(w3, dtype=np.float32)

    half = D // 2
    inv_freq = 1.0 / (ROPE_BASE ** (np.arange(half, dtype=np.float32) / half))
    pos = np.arange(T, dtype=np.float32)
    freqs = pos[:, None] * inv_freq[None, :]
    rope_tab = np.ascontiguousarray(
        np.concatenate([np.cos(freqs).T, np.sin(freqs).T], axis=0).astype(np.float32)
    )

    ql = np.arange(P)[None, :]
    kv = np.arange(P)[:, None]
    tri = (ql >= kv).astype(NP_MMDT)

    # fold w_norm into weight rows (h @ W == (x*rstd) @ (diag(wn) W))
    w_qkv_n = w_qkv * w_norm1[:, None]
    w1_n = w1 * w_norm2[:, None]
    w2_n = w2 * w_norm2[:, None]

    # [HID_T, P, CT*P]: w1t[ht, p, ct*P + d] = w1_n[ct*P + p, ht*P + d]
    w1t = np.ascontiguousarray(
        w1_n.reshape(CT, P, HID_T, P).transpose(2, 1, 0, 3).reshape(HID_T, P, C)
    ).astype(NP_MMDT)
    w2t = np.ascontiguousarray(
        w2_n.reshape(CT, P, HID_T, P).transpose(2, 1, 0, 3).reshape(HID_T, P, C)
    ).astype(NP_MMDT)
    # [CT, P, HID_T*P]: w3r[ct, p, ht*P + d] = w3[ht*P + p, ct*P + d]
    w3r_h = np.ascontiguousarray(
        w3.reshape(HID_T, P, CT, P).transpose(2, 1, 0, 3).reshape(CT, P, HID)
    ).astype(NP_MMDT)

    # [P, CT, cols]: wqkv_r[p, ct, d] = w_qkv_n[ct*P + p, d]
    wqkv_r = np.ascontiguousarray(
        w_qkv_n.reshape(CT, P, 3 * C).transpose(1, 0, 2)
    ).astype(NP_MMDT)

    # wpe: [2, CT, P, 8*P].  Pass p block s = w_proj rows of head (2s + p)
    # (receiver slot s carries head 2s for pass 0, head 2s+1 for pass 1).
    wpe_full = np.empty((2, 8, P, C), dtype=np.float32)
    for s_ in range(8):
        wpe_full[0, s_] = w_proj[(2 * s_) * P : (2 * s_ + 1) * P, :]
        wpe_full[1, s_] = w_proj[(2 * s_ + 1) * P : (2 * s_ + 2) * P, :]
    wpe_r_h = np.ascontiguousarray(
        wpe_full.reshape(2, 8, P, CT, P).transpose(0, 3, 2, 1, 4).reshape(2, CT, P, 8 * P)
    ).astype(NP_MMDT)

    # x_t: [C, 2T] both batches, bf16 (shared across cores)
    x_t_h = np.ascontiguousarray(
        np.concatenate([x[0].T, x[1].T], axis=1)
    ).astype(NP_MMDT)

    in_maps = []
    for j in range(8):
        b, hg = j // 4, j % 4
        col0 = 2 * j * D  # first head of this core's pair
        in_maps.append(
            {
                "x_t": x_t_h,
                "wq": np.ascontiguousarray(
                    wqkv_r[:, :, col0 : col0 + HPC * D]
                    .reshape(P, CT, HPC, P)
                    .transpose(2, 0, 1, 3)
                    .reshape(HPC, P, C)
                ),
                "wk": np.ascontiguousarray(
                    wqkv_r[:, :, C + col0 : C + col0 + HPC * D]
                    .reshape(P, CT, HPC, P)
                    .transpose(2, 0, 1, 3)
                    .reshape(HPC, P, C)
                ),
                "wv": np.ascontiguousarray(
                    wqkv_r[:, :, 2 * C + col0 : 2 * C + col0 + HPC * D]
                ),
                "wpe_r": wpe_r_h,
                "w1t": w1t,
                "w2t": w2t,
                "w3r": w3r_h,
                "rope_t": rope_tab,
                "tri": tri,
                "x_res": np.ascontiguousarray(
                    x_t_h[:, b * T + hg * TQ : b * T + (hg + 1) * TQ]
                ),
            }
        )
    return in_maps


def kernel(x, w_norm1, w_qkv, w_proj, w_norm2, w1, w2, w3, _trace=False, _tmpdir=None):
    nc = _get_nc()
    in_maps = _host_inputs(x, w_norm1, w_qkv, w_proj, w_norm2, w1, w2, w3)
    kwargs = {}
    if _trace:
        kwargs = {"trace": True, "tmpdir": _tmpdir}
    res = bass_utils.run_bass_kernel_spmd(
        nc, in_maps, core_ids=list(range(8)), **kwargs
    )
    out = np.empty((2, T, C), dtype=np.float32)
    for j in range(8):
        out[j // 4, (j % 4) * TQ : (j % 4 + 1) * TQ, :] = res.results[j]["out"].T
    kernel._last_exec_time_ns = res.exec_time_ns
    return out


# revision 31
# speedup vs baseline: 1.1225x; 1.0058x over previous
"""Dense transformer block (rmsnorm+causal attention+rope / rmsnorm+SwiGLU) on 8 TRN2 cores.

Sharding v2:
  core j: heads {2j, 2j+1} x BOTH batches (4 head-instances).  Every core
  therefore holds attention output destined for all 8 phase-B owners, so the
  two AllToAlls carry fully-dense payload (no cross-batch zero slots, no
  bmask, no receiver slot-pair sums).  A2A1 fires after the even-head
  instances (50% of attention), A2A2 after the odd-head instances; proj
  pass 0 overlaps A2A2's flight.
  Phase B owner of core j: batch j//4, token block j%4 (as baseline).

  Phase A: rmsnorm1 stats via ScalarE squares + DVE ct-accumulation + one
  ones-matmul per chunk; q/k/v matmuls run on RAW x (rstd folded into the
  rope tables for q/k, transposed per-row scale for v) so the PE never
  waits on the stats chain.  Causal attention keeps q/k/v SBUF-resident;
  softmax denominators are DVE-accumulated (bf16 eacc, interleaved with
  exp) + two 512-col ones-matmuls emitted after the AV pass.
  Phase B runs fully transposed: proj y^T accumulates [C, TQ], residual
  read straight from x_t (bf16), rmsnorm2 + SwiGLU with a transposed w3
  pass; output is [C, TQ] per core, transposed on host.

Matmul operands are bf16 (weights and x pre-cast on host, w_norm folded into
weight rows); statistics, softmax sums, residual accumulation and PSUM stay
fp32.
"""

import numpy as np
import ml_dtypes

import concourse.bass as bass
import concourse.mybir as mybir
import concourse.tile as tile
from concourse import bacc
from concourse import bass_utils
from concourse.masks import make_identity

AF = mybir.ActivationFunctionType
ALU = mybir.AluOpType
F32 = mybir.dt.float32
BF16 = mybir.dt.bfloat16
MMDT = BF16
NP_MMDT = ml_dtypes.bfloat16

P = 128
T = 2048
C = 2048
D = 128
H = 16
HPC = 2          # heads per core
NB = 2           # batches (all processed on every core)
HID = 5632
HID_T = HID // P  # 44 hid tiles
TQ = 512         # A2A / output col-block granularity
TQA = 1024       # attention query-chunk
EPS = 1e-6
ROPE_BASE = 10000.0
CT = C // P      # 16 contraction tiles
HD2 = D // 2


def _build():
    nc = bacc.Bacc(None, target_bir_lowering=False, num_devices=8)

    # ---- kernel I/O ----
    x_t = nc.dram_tensor("x_t", [C, NB * T], MMDT, kind="ExternalInput")
    wq = nc.dram_tensor("wq", [HPC, P, CT * P], MMDT, kind="ExternalInput")
    wk = nc.dram_tensor("wk", [HPC, P, CT * P], MMDT, kind="ExternalInput")
    wv = nc.dram_tensor("wv", [P, CT, HPC * D], MMDT, kind="ExternalInput")
    wpe_r = nc.dram_tensor("wpe_r", [2, CT, P, 8 * P], MMDT, kind="ExternalInput")
    w1t = nc.dram_tensor("w1t", [HID_T, P, CT * P], MMDT, kind="ExternalInput")
    w2t = nc.dram_tensor("w2t", [HID_T, P, CT * P], MMDT, kind="ExternalInput")
    w3r = nc.dram_tensor("w3r", [CT, P, HID_T * P], MMDT, kind="ExternalInput")
    rope_t = nc.dram_tensor("rope_t", [D, T], F32, kind="ExternalInput")
    tri = nc.dram_tensor("tri", [P, P], MMDT, kind="ExternalInput")
    x_res = nc.dram_tensor("x_res", [C, TQ], MMDT, kind="ExternalInput")
    out = nc.dram_tensor("out", [C, TQ], F32, kind="ExternalOutput")

    inv_sqrt_d = 1.0 / float(np.sqrt(D))
    GROUPS = [[0, 1, 2, 3, 4, 5, 6, 7]]

    with tile.TileContext(nc) as tc:
        with (
            tc.tile_pool(name="const", bufs=1) as const,
            tc.tile_pool(name="dram", bufs=1, space="DRAM") as dram,
        ):
            # ---- constants ----
            ones_f = const.tile([P, 1], F32)
            nc.vector.memset(ones_f, 1.0)
            ones_r = const.tile([P, 1], MMDT)
            nc.vector.tensor_copy(out=ones_r, in_=ones_f)
            eps_sb = const.tile([P, 1], F32)
            nc.vector.memset(eps_sb, EPS)
            ident_f = const.tile([P, P], F32)
            make_identity(nc, ident_f)
            rope_sb = const.tile([D, T], F32)
            tri_sb = const.tile([P, P], MMDT)

            # ---- DRAM scratch for collectives ----
            a2a1_in = dram.tile([8, P, TQ], MMDT)
            a2a1_out = dram.tile([8, P, TQ], MMDT)
            a2a2_in = dram.tile([8, P, TQ], MMDT)
            a2a2_out = dram.tile([8, P, TQ], MMDT)

            # ---- persistent SBUF: first MLP weight tiles (filled during
            # attention while the DMA engines are otherwise idle) ----
            NPRE = 3
            wpre_ctx = tc.tile_pool(name="wpre", bufs=1)
            wpre = wpre_ctx.__enter__()
            w1pre = wpre.tile([P, NPRE, CT, P], MMDT, tag="w1pre", bufs=1)
            w2pre = wpre.tile([P, NPRE, CT, P], MMDT, tag="w2pre", bufs=1)

            # ---- persistent SBUF across phase A (q/k/v resident) ----
            # instance index: inst = 2*m + b  (m: head-in-pair, b: batch)
            qkvp_ctx = tc.tile_pool(name="qkvp", bufs=1)
            qkvp = qkvp_ctx.__enter__()
            qT_sb = qkvp.tile([P, 2 * HPC, T], MMDT, tag="qT_sb", bufs=1)
            kT_sb = qkvp.tile([P, 2 * HPC, T], MMDT, tag="kT_sb", bufs=1)
            v_sb = qkvp.tile([P, NB * T // P, HPC * D], MMDT, tag="v_sb", bufs=1)

            # ================= Phase A1+A2: rmsnorm1 + QKV (chunked) ============
            with (
                tc.tile_pool(name="p12", bufs=2) as p12,
                tc.tile_pool(name="p12psum", bufs=2, space="PSUM") as pp12,
            ):
                CHUNKS = [256, 256, 512, 512, 512]  # per batch (sums to T)
                wq_sb = p12.tile([P, HPC, CT, P], MMDT, tag="wq_sb", bufs=1)
                wk_sb = p12.tile([P, HPC, CT, P], MMDT, tag="wk_sb", bufs=1)
                nc.sync.dma_start(out=wq_sb[:, 0], in_=wq[0])
                first_xt = p12.tile([P, CT, CHUNKS[0]], MMDT, tag="xt256", bufs=2)
                nc.sync.dma_start(
                    out=first_xt,
                    in_=x_t[:, 0 : CHUNKS[0]].rearrange("(ct p) t -> p ct t", p=P),
                )
                for m_ in range(1, HPC):
                    nc.sync.dma_start(out=wq_sb[:, m_], in_=wq[m_])
                for m_ in range(HPC):
                    nc.sync.dma_start(out=wk_sb[:, m_], in_=wk[m_])
                wv_sb = p12.tile([P, CT, HPC * D], MMDT, tag="wv_sb", bufs=1)
                nc.sync.dma_start(out=wv_sb, in_=wv[:, :, :])
                nc.sync.dma_start(out=rope_sb, in_=rope_t[:, :])
                nc.sync.dma_start(out=tri_sb, in_=tri[:, :])

                for b in range(NB):
                    t0p = 0  # within-batch position
                    for ch, CHW in enumerate(CHUNKS):
                        t0 = b * T + t0p  # global column in x_t / v_sb
                        if b == 0 and ch == 0:
                            xt = first_xt
                        else:
                            xt = p12.tile(
                                [P, CT, CHW], MMDT, tag=f"xt{CHW}", bufs=2, name="xt"
                            )
                            nc.sync.dma_start(
                                out=xt,
                                in_=x_t[:, t0 : t0 + CHW].rearrange(
                                    "(ct p) t -> p ct t", p=P
                                ),
                            )
                        # rmsnorm stats: squares on ScalarE, partition-sum on PE
                        sq = p12.tile([P, CT, CHW], MMDT, tag=f"sq{CHW}", bufs=1, name="sq")
                        nc.scalar.activation(
                            sq.rearrange("p a b -> p (a b)"),
                            xt.rearrange("p a b -> p (a b)"),
                            AF.Square,
                        )
                        ssum = pp12.tile([1, TQ], F32, tag="ssum", bufs=1, name="ssum")[:, :CHW]
                        for ct in range(CT):
                            nc.tensor.matmul(
                                ssum,
                                ones_r,
                                sq[:, ct, :],
                                start=(ct == 0),
                                stop=(ct == CT - 1),
                            )
                        srow = p12.tile([1, TQ], F32, tag="srow", bufs=2, name="srow")[:, :CHW]
                        nc.scalar.activation(
                            srow, ssum, AF.Sqrt, bias=eps_sb[0:1, :], scale=1.0 / C
                        )
                        rstd_row = p12.tile([1, TQ], F32, tag="rstd_row", bufs=2, name="rstd_row")[:, :CHW]
                        nc.vector.reciprocal_approx_fast(out=rstd_row, in_=srow)
                        rstd_bc = p12.tile([P, TQ], F32, tag="rstd_bc", bufs=2, name="rstd_bc")[:, :CHW]
                        nc.gpsimd.partition_broadcast(rstd_bc[:], rstd_row[:])
                        # rstd folded into rope tables: rows 0:64 cos*rstd,
                        # 64:128 sin*rstd (q/k matmuls run on RAW x so the PE
                        # never waits on the stats chain)
                        cs_r = p12.tile([P, TQ], F32, tag="cs_r", bufs=2, name="cs_r")[:, :CHW]
                        nc.vector.tensor_tensor(
                            out=cs_r, in0=rope_sb[:, t0p : t0p + CHW], in1=rstd_bc,
                            op=ALU.mult,
                        )

                        # q^T / k^T with fused rope(+rstd) on eviction
                        for w_sb, dst in ((wq_sb, qT_sb), (wk_sb, kT_sb)):
                            for m in range(HPC):
                                inst = 2 * m + b
                                pq = pp12.tile([P, TQ], F32, tag="qk", bufs=4, name="pq")[:, :CHW]
                                for ct in range(CT):
                                    nc.tensor.matmul(
                                        pq,
                                        w_sb[:, m, ct, :],
                                        xt[:, ct, :],
                                        start=(ct == 0),
                                        stop=(ct == CT - 1),
                                    )
                                x1 = pq[0:HD2, :]
                                x2 = pq[HD2:P, :]
                                cosw = cs_r[0:HD2, :]
                                sinw = cs_r[HD2:P, :]
                                tm1 = p12.tile([HD2, TQ], F32, tag="tm1", bufs=2, name="tm1")[:, :CHW]
                                tm2 = p12.tile([HD2, TQ], F32, tag="tm2", bufs=2, name="tm2")[:, :CHW]
                                nc.vector.tensor_tensor(out=tm1, in0=x1, in1=cosw, op=ALU.mult)
                                nc.vector.tensor_tensor(out=tm2, in0=x2, in1=sinw, op=ALU.mult)
                                nc.vector.tensor_tensor(
                                    out=dst[0:HD2, inst, t0p : t0p + CHW],
                                    in0=tm1,
                                    in1=tm2,
                                    op=ALU.subtract,
                                )
                                nc.vector.tensor_tensor(out=tm1, in0=x1, in1=sinw, op=ALU.mult)
                                nc.vector.tensor_tensor(out=tm2, in0=x2, in1=cosw, op=ALU.mult)
                                nc.vector.tensor_tensor(
                                    out=dst[HD2:P, inst, t0p : t0p + CHW],
                                    in0=tm1,
                                    in1=tm2,
                                    op=ALU.add,
                                )

                        # v in row layout [t, 2*D]; per-row rstd via PE-transposed col
                        for rt in range(CHW // P):
                            trp = pp12.tile([P, P], F32, tag="trp", bufs=1)
                            nc.tensor.transpose(
                                trp, rstd_bc[:, rt * P : (rt + 1) * P], ident_f
                            )
                            rstd_col = p12.tile([P, 1], F32, tag="rstd_col", bufs=2)
                            nc.vector.tensor_copy(out=rstd_col, in_=trp[:, 0:1])
                            pv = pp12.tile([P, HPC * D], F32, tag="v", bufs=2)
                            for ct in range(CT):
                                nc.tensor.matmul(
                                    pv,
                                    xt[:, ct, rt * P : (rt + 1) * P],
                                    wv_sb[:, ct, :],
                                    start=(ct == 0),
                                    stop=(ct == CT - 1),
                                )
                            nc.vector.tensor_scalar(
                                out=v_sb[:, t0 // P + rt, :],
                                in0=pv,
                                scalar1=rstd_col,
                                scalar2=None,
                                op0=ALU.mult,
                            )
                        t0p += CHW

            # prefetch the first MLP weight tiles during attention (DMA idle)
            for hp in range(NPRE):
                nc.sync.dma_start(out=w1pre[:, hp], in_=w1t[hp])
            for hp in range(NPRE):
                nc.sync.dma_start(out=w2pre[:, hp], in_=w2t[hp])

            # ================= Phase A3: causal attention (+ A2As) ==============
            with (
                tc.tile_pool(name="att", bufs=2) as att,
                tc.tile_pool(name="attpsum", bufs=2, space="PSUM") as pat,
            ):
                for m in range(HPC):
                    a2a_in = (a2a1_in, a2a2_in)[m]
                    for b in range(NB):
                        inst = 2 * m + b
                        for q2 in range(T // TQA):
                            qb = q2 * TQA
                            o_ps = pat.tile([P, TQA], F32, tag="o", bufs=1)
                            es = []
                            n_items = 8 * q2 + 8
                            # last writer of bank0 (cols 0:TQ) is the r=3 diag
                            # item; last writer of bank1 is the final item
                            b0_last = n_items - 5

                            def av_emit(idx):
                                kb, q0, e = es[idx]
                                first = idx == 0
                                if q0 < TQ:
                                    nc.tensor.matmul(
                                        o_ps[:, q0:TQ],
                                        v_sb[:, b * (T // P) + kb, m * D : (m + 1) * D],
                                        e[:, q0:TQ],
                                        start=first, stop=(idx == b0_last),
                                    )
                                    nc.tensor.matmul(
                                        o_ps[:, TQ:TQA],
                                        v_sb[:, b * (T // P) + kb, m * D : (m + 1) * D],
                                        e[:, TQ:TQA],
                                        start=first, stop=(idx == n_items - 1),
                                    )
                                else:
                                    nc.tensor.matmul(
                                        o_ps[:, q0:TQA],
                                        v_sb[:, b * (T // P) + kb, m * D : (m + 1) * D],
                                        e[:, q0:TQA],
                                        start=first, stop=(idx == n_items - 1),
                                    )
                            # full key blocks
                            for kb in range(8 * q2):
                                st = pat.tile([P, TQA], F32, tag="st", bufs=2)
                                for i in range(2):
                                    nc.tensor.matmul(
                                        st[:, i * TQ : (i + 1) * TQ],
                                        kT_sb[:, inst, kb * P : (kb + 1) * P],
                                        qT_sb[:, inst, qb + i * TQ : qb + (i + 1) * TQ],
                                        start=True,
                                        stop=True,
                                    )
                                e = att.tile([P, TQA], MMDT, tag="e", bufs=18)
                                nc.scalar.activation(e, st, AF.Exp, scale=inv_sqrt_d)
                                if not es:
                                    eacc = att.tile([P, TQA], MMDT, tag="eacc", bufs=2)
                                    nc.vector.tensor_copy(out=eacc, in_=e)
                                else:
                                    nc.vector.tensor_tensor(
                                        out=eacc, in0=eacc, in1=e, op=ALU.add
                                    )
                                es.append((kb, 0, e))
                                if len(es) >= 4:
                                    av_emit(len(es) - 4)
                            # diagonal blocks (r = 0..7), masked region trimmed
                            for r in range(8):
                                kb = 8 * q2 + r
                                q0 = r * P
                                st = pat.tile([P, TQA], F32, tag="st", bufs=2)
                                if q0 < TQ:
                                    nc.tensor.matmul(
                                        st[:, q0:TQ],
                                        kT_sb[:, inst, kb * P : (kb + 1) * P],
                                        qT_sb[:, inst, qb + q0 : qb + TQ],
                                        start=True,
                                        stop=True,
                                    )
                                    nc.tensor.matmul(
                                        st[:, TQ:TQA],
                                        kT_sb[:, inst, kb * P : (kb + 1) * P],
                                        qT_sb[:, inst, qb + TQ : qb + TQA],
                                        start=True,
                                        stop=True,
                                    )
                                else:
                                    nc.tensor.matmul(
                                        st[:, q0:TQA],
                                        kT_sb[:, inst, kb * P : (kb + 1) * P],
                                        qT_sb[:, inst, qb + q0 : qb + TQA],
                                        start=True,
                                        stop=True,
                                    )
                                e = att.tile([P, TQA], MMDT, tag="e", bufs=18)
                                nc.scalar.activation(
                                    e[:, q0:TQA], st[:, q0:TQA], AF.Exp, scale=inv_sqrt_d
                                )
                                nc.vector.tensor_tensor(
                                    out=e[:, q0 : q0 + P],
                                    in0=e[:, q0 : q0 + P],
                                    in1=tri_sb,
                                    op=ALU.mult,
                                )
                                if not es:
                                    eacc = att.tile([P, TQA], MMDT, tag="eacc", bufs=2)
                                    nc.vector.tensor_copy(out=eacc, in_=e)
                                else:
                                    nc.vector.tensor_tensor(
                                        out=eacc[:, q0:TQA],
                                        in0=eacc[:, q0:TQA],
                                        in1=e[:, q0:TQA],
                                        op=ALU.add,
                                    )
                                es.append((kb, q0, e))
                                if len(es) >= 4:
                                    av_emit(len(es) - 4)
                            # drain the AV pipeline, then the denominator
                            # matmuls (their recip/bcast/oT chain overlaps the
                            # next group's QK pass)
                            for idx in range(n_items - 3, n_items):
                                av_emit(idx)
                            l_ps = pat.tile([1, TQA], F32, tag="l", bufs=1)
                            nc.tensor.matmul(
                                l_ps[:, 0:TQ], ones_r, eacc[:, 0:TQ],
                                start=True, stop=True,
                            )
                            nc.tensor.matmul(
                                l_ps[:, TQ:TQA], ones_r, eacc[:, TQ:TQA],
                                start=True, stop=True,
                            )
                            l_inv = att.tile([1, TQA], F32, tag="l_inv", bufs=2)
                            nc.vector.reciprocal_approx_fast(out=l_inv, in_=l_ps)
                            l_bc = att.tile([P, TQA], F32, tag="l_bc", bufs=2)
                            nc.gpsimd.partition_broadcast(l_bc[:], l_inv[:])
                            oT = att.tile([P, TQA], MMDT, tag="oT", bufs=2)
                            nc.vector.tensor_tensor(out=oT, in0=o_ps, in1=l_bc, op=ALU.mult)
                            for i in range(2):
                                qc = 2 * q2 + i  # phase-B token block 0..3
                                nc.sync.dma_start(
                                    out=a2a_in[b * 4 + qc, :, :],
                                    in_=oT[:, i * TQ : (i + 1) * TQ],
                                )
                    if m == 0:
                        nc.gpsimd.collective_compute(
                            "AllToAll",
                            ALU.bypass,
                            replica_groups=GROUPS,
                            ins=[a2a1_in.opt()],
                            outs=[a2a1_out.opt()],
                        )
            qkvp_ctx.__exit__(None, None, None)

            # A2A2 emitted outside the attention pool so its completion doesn't
            # gate the pool-close barrier; it overlaps proj pass 0.
            nc.gpsimd.collective_compute(
                "AllToAll",
                ALU.bypass,
                replica_groups=GROUPS,
                ins=[a2a2_in.opt()],
                outs=[a2a2_out.opt()],
            )

            # ---- persistent SBUF through phase B ----
            bper_ctx = tc.tile_pool(name="bper", bufs=1)
            bper = bper_ctx.__enter__()
            xmidT = bper.tile([P, CT, TQ], F32, tag="xmidT", bufs=1)
            h2T = bper.tile([P, CT, TQ], MMDT, tag="h2T", bufs=1)

            # ========== Phase B1: proj^T + residual + rmsnorm2 (transposed) ======
            with (
                tc.tile_pool(name="proj", bufs=2) as prj,
                tc.tile_pool(name="projpsum", bufs=2, space="PSUM") as ppj,
            ):
                lp0 = prj.tile([P, 8, TQ], MMDT, tag="lp0", bufs=1)
                lp1 = prj.tile([P, 8, TQ], MMDT, tag="lp1", bufs=1)
                wpe_first = prj.tile([P, 8, P], MMDT, tag="wpe_first", bufs=1)
                nc.sync.dma_start(out=wpe_first, in_=wpe_r[0, 0, :, 0 : 8 * P])
                # lp0[p, s, t] = a2a1_out[s, p, t]: slot s = head 2s, one DMA.
                nc.sync.dma_start(
                    out=lp0, in_=a2a1_out.rearrange("s p t -> p s t")
                )
                # pass 0: even heads (a2a1), into xmidT
                for ct in range(CT):
                    if ct == 0:
                        wpe_sb = wpe_first
                    else:
                        wpe_sb = prj.tile([P, 8, P], MMDT, tag="wpe_sb", bufs=3)
                        nc.sync.dma_start(out=wpe_sb, in_=wpe_r[0, ct, :, 0 : 8 * P])
                    yps = ppj.tile([P, TQ], F32, tag="y", bufs=4)
                    for blk in range(8):
                        nc.tensor.matmul(
                            yps,
                            wpe_sb[:, blk, :],
                            lp0[:, blk, :],
                            start=(blk == 0),
                            stop=(blk == 7),
                        )
                    nc.scalar.copy(out=xmidT[:, ct, :], in_=yps)
                # residual slice of x (bf16, per-core input)
                xres = prj.tile([P, CT, TQ], MMDT, tag="xres", bufs=1)
                nc.sync.dma_start(
                    out=xres,
                    in_=x_res[:, :].rearrange("(ct p) t -> p ct t", p=P),
                )
                # pass 1: odd heads (a2a2) + residual, rmsnorm2 stats per ct
                nc.sync.dma_start(
                    out=lp1, in_=a2a2_out.rearrange("s p t -> p s t")
                )
                sq2 = bper.tile([P, CT, TQ], MMDT, tag="sq2", bufs=1)
                ssum2 = ppj.tile([1, TQ], F32, tag="ssum2", bufs=1)
                for ct in range(CT):
                    wpe_sb1 = prj.tile([P, 8, P], MMDT, tag="wpe_sb1", bufs=3)
                    nc.sync.dma_start(
                        out=wpe_sb1, in_=wpe_r[1, ct, :, 0 : 8 * P]
                    )
                    yps = ppj.tile([P, TQ], F32, tag="y", bufs=4)
                    for blk in range(8):
                        nc.tensor.matmul(
                            yps,
                            wpe_sb1[:, blk, :],
                            lp1[:, blk, :],
                            start=(blk == 0),
                            stop=(blk == 7),
                        )
                    t1 = prj.tile([P, TQ], F32, tag="t1", bufs=3)
                    nc.vector.tensor_tensor(
                        out=t1, in0=yps, in1=xmidT[:, ct, :], op=ALU.add
                    )
                    nc.vector.tensor_tensor(
                        out=xmidT[:, ct, :], in0=t1, in1=xres[:, ct, :], op=ALU.add
                    )
                    nc.scalar.activation(
                        sq2[:, ct, :], xmidT[:, ct, :], AF.Square
                    )
                    nc.tensor.matmul(
                        ssum2, ones_r, sq2[:, ct, :], start=(ct == 0), stop=(ct == CT - 1)
                    )
                srow2 = bper.tile([1, TQ], F32, tag="srow2", bufs=1)
                nc.scalar.activation(
                    srow2, ssum2, AF.Sqrt, bias=eps_sb[0:1, :], scale=1.0 / C
                )
                rstd2 = bper.tile([1, TQ], F32, tag="rstd2", bufs=1)
                nc.vector.reciprocal_approx_fast(out=rstd2, in_=srow2)
                rstd2_bc = bper.tile([P, TQ], F32, tag="rstd2_bc", bufs=1)
                nc.gpsimd.partition_broadcast(rstd2_bc[:], rstd2[:])
                for ct in range(CT):
                    nc.vector.tensor_tensor(
                        out=h2T[:, ct, :], in0=xmidT[:, ct, :], in1=rstd2_bc, op=ALU.mult
                    )

            # ================= Phase B2: SwiGLU (transposed w3 pass) =============
            with (
                tc.tile_pool(name="mlp", bufs=2) as mlp,
                tc.tile_pool(name="mlppsum", bufs=2, space="PSUM") as pml,
            ):
                uT = mlp.tile([P, HID_T, TQ], MMDT, tag="uT", bufs=1)
                for ht in range(HID_T):
                    if ht < NPRE:
                        w1_sb = w1pre[:, ht]
                        w2_sb = w2pre[:, ht]
                    else:
                        w1_sb = mlp.tile([P, CT, P], MMDT, tag="w1_sb", bufs=3)
                        nc.sync.dma_start(out=w1_sb, in_=w1t[ht])
                        w2_sb = mlp.tile([P, CT, P], MMDT, tag="w2_sb", bufs=3)
                        nc.sync.dma_start(out=w2_sb, in_=w2t[ht])
                    g1 = pml.tile([P, TQ], F32, tag="g1", bufs=2)
                    g2 = pml.tile([P, TQ], F32, tag="g2", bufs=2)
                    for ct in range(CT):
                        nc.tensor.matmul(
                            g1, w1_sb[:, ct, :], h2T[:, ct, :],
                            start=(ct == 0), stop=(ct == CT - 1),
                        )
                    for ct in range(CT):
                        nc.tensor.matmul(
                            g2, w2_sb[:, ct, :], h2T[:, ct, :],
                            start=(ct == 0), stop=(ct == CT - 1),
                        )
                    sil = mlp.tile([P, TQ], F32, tag="sil", bufs=3)
                    nc.scalar.activation(sil, g1, AF.Silu)
                    nc.vector.tensor_tensor(
                        out=uT[:, ht, :], in0=g2, in1=sil, op=ALU.mult
                    )
                # y3^T: stationary w3 blocks, moving uT; accumulate 44 ht per ct
                for ct in range(CT):
                    w3_sb = mlp.tile([P, HID_T, P], MMDT, tag="w3_sb", bufs=2)
                    nc.sync.dma_start(out=w3_sb, in_=w3r[ct])
                    y3 = pml.tile([P, TQ], F32, tag="y3", bufs=2)
                    for ht in range(HID_T):
                        nc.tensor.matmul(
                            y3, w3_sb[:, ht, :], uT[:, ht, :],
                            start=(ht == 0), stop=(ht == HID_T - 1),
                        )
                    ofin = mlp.tile([P, TQ], F32, tag="ofin", bufs=3)
                    nc.vector.tensor_tensor(
                        out=ofin, in0=y3, in1=xmidT[:, ct, :], op=ALU.add
                    )
                    nc.sync.dma_start(out=out[ct * P : (ct + 1) * P, :], in_=ofin)
            bper_ctx.__exit__(None, None, None)
            wpre_ctx.__exit__(None, None, None)

    nc.compile()
    return nc


_NC_CACHE = None


def _get_nc():
    global _NC_CACHE
    if _NC_CACHE is None:
        _NC_CACHE = _build()
    return _NC_CACHE


def _host_inputs(x, w_norm1, w_qkv, w_proj, w_norm2, w1, w2, w3):
    x = np.asarray(x, dtype=np.float32)
    w_qkv = np.asarray(w_qkv, dtype=np.float32)
    w_proj = np.asarray(w_proj, dtype=np.float32)
    w_norm1 = np.asarray(w_norm1, dtype=np.float32)
    w_norm2 = np.asarray(w_norm2, dtype=np.float32)
    w1 = np.asarray(w1, dtype=np.float32)
    w2 = np.asarray(w2, dtype=np.float32)
    w3 = np.asarray(w3, dtype=np.float32)

    half = D // 2
    inv_freq = 1.0 / (ROPE_BASE ** (np.arange(half, dtype=np.float32) / half))
    pos = np.arange(T, dtype=np.float32)
    freqs = pos[:, None] * inv_freq[None, :]
    rope_tab = np.ascontiguousarray(
        np.concatenate([np.cos(freqs).T, np.sin(freqs).T], axis=0).astype(np.float32)
    )

    ql = np.arange(P)[None, :]
    kv = np.arange(P)[:, None]
    tri = (ql >= kv).astype(NP_MMDT)

    # fold w_norm into weight rows (h @ W == (x*rstd) @ (diag(wn) W))
    w_qkv_n = w_qkv * w_norm1[:, None]
    w1_n = w1 * w_norm2[:, None]
    w2_n = w2 * w_norm2[:, None]

    # [HID_T, P, CT*P]: w1t[ht, p, ct*P + d] = w1_n[ct*P + p, ht*P + d]
    w1t = np.ascontiguousarray(
        w1_n.reshape(CT, P, HID_T, P).transpose(2, 1, 0, 3).reshape(HID_T, P, C)
    ).astype(NP_MMDT)
    w2t = np.ascontiguousarray(
        w2_n.reshape(CT, P, HID_T, P).transpose(2, 1, 0, 3).reshape(HID_T, P, C)
    ).astype(NP_MMDT)
    # [CT, P, HID_T*P]: w3r[ct, p, ht*P + d] = w3[ht*P + p, ct*P + d]
    w3r_h = np.ascontiguousarray(
        w3.reshape(HID_T, P, CT, P).transpose(2, 1, 0, 3).reshape(CT, P, HID)
    ).astype(NP_MMDT)

    # [P, CT, cols]: wqkv_r[p, ct, d] = w_qkv_n[ct*P + p, d]
    wqkv_r = np.ascontiguousarray(
        w_qkv_n.reshape(CT, P, 3 * C).transpose(1, 0, 2)
    ).astype(NP_MMDT)

    # wpe: [2, CT, P, 8*P].  Pass p block s = w_proj rows of head (2s + p)
    # (receiver slot s carries head 2s for pass 0, head 2s+1 for pass 1).
    wpe_full = np.empty((2, 8, P, C), dtype=np.float32)
    for s_ in range(8):
        wpe_full[0, s_] = w_proj[(2 * s_) * P : (2 * s_ + 1) * P, :]
        wpe_full[1, s_] = w_proj[(2 * s_ + 1) * P : (2 * s_ + 2) * P, :]
    wpe_r_h = np.ascontiguousarray(
        wpe_full.reshape(2, 8, P, CT, P).transpose(0, 3, 2, 1, 4).reshape(2, CT, P, 8 * P)
    ).astype(NP_MMDT)

    # x_t: [C, 2T] both batches, bf16 (shared across cores)
    x_t_h = np.ascontiguousarray(
        np.concatenate([x[0].T, x[1].T], axis=1)
    ).astype(NP_MMDT)

    in_maps = []
    for j in range(8):
        b, hg = j // 4, j % 4
        col0 = 2 * j * D  # first head of this core's pair
        in_maps.append(
            {
                "x_t": x_t_h,
                "wq": np.ascontiguousarray(
                    wqkv_r[:, :, col0 : col0 + HPC * D]
                    .reshape(P, CT, HPC, P)
                    .transpose(2, 0, 1, 3)
                    .reshape(HPC, P, C)
                ),
                "wk": np.ascontiguousarray(
                    wqkv_r[:, :, C + col0 : C + col0 + HPC * D]
                    .reshape(P, CT, HPC, P)
                    .transpose(2, 0, 1, 3)
                    .reshape(HPC, P, C)
                ),
                "wv": np.ascontiguousarray(
                    wqkv_r[:, :, 2 * C + col0 : 2 * C + col0 + HPC * D]
                ),
                "wpe_r": wpe_r_h,
                "w1t": w1t,
                "w2t": w2t,
                "w3r": w3r_h,
                "rope_t": rope_tab,
                "tri": tri,
                "x_res": np.ascontiguousarray(
                    x_t_h[:, b * T + hg * TQ : b * T + (hg + 1) * TQ]
                ),
            }
        )
    return in_maps


def kernel(x, w_norm1, w_qkv, w_proj, w_norm2, w1, w2, w3, _trace=False, _tmpdir=None):
    nc = _get_nc()
    in_maps = _host_inputs(x, w_norm1, w_qkv, w_proj, w_norm2, w1, w2, w3)
    kwargs = {}
    if _trace:
        kwargs = {"trace": True, "tmpdir": _tmpdir}
    res = bass_utils.run_bass_kernel_spmd(
        nc, in_maps, core_ids=list(range(8)), **kwargs
    )
    out = np.empty((2, T, C), dtype=np.float32)
    for j in range(8):
        out[j // 4, (j % 4) * TQ : (j % 4 + 1) * TQ, :] = res.results[j]["out"].T
    kernel._last_exec_time_ns = res.exec_time_ns
    return out


# revision 34
# speedup vs baseline: 1.1289x; 1.0058x over previous
"""Dense transformer block (rmsnorm+causal attention+rope / rmsnorm+SwiGLU) on 8 TRN2 cores.

Sharding v2:
  core j: heads {2j, 2j+1} x BOTH batches (4 head-instances).  Every core
  therefore holds attention output destined for all 8 phase-B owners, so the
  two AllToAlls carry fully-dense payload (no cross-batch zero slots, no
  bmask, no receiver slot-pair sums).  A2A1 fires after the even-head
  instances (50% of attention), A2A2 after the odd-head instances; proj
  pass 0 overlaps A2A2's flight.
  Phase B owner of core j: batch j//4, token block j%4 (as baseline).

  Phase A: rmsnorm1 stats via ScalarE squares + DVE ct-accumulation + one
  ones-matmul per chunk; q/k/v matmuls run on RAW x (rstd folded into the
  rope tables for q/k, transposed per-row scale for v) so the PE never
  waits on the stats chain.  Causal attention keeps q/k/v SBUF-resident;
  softmax denominators are DVE-accumulated (bf16 eacc, interleaved with
  exp) + two 512-col ones-matmuls emitted after the AV pass.
  Phase B runs fully transposed: proj y^T accumulates [C, TQ], residual
  read straight from x_t (bf16), rmsnorm2 + SwiGLU with a transposed w3
  pass; output is [C, TQ] per core, transposed on host.

Matmul operands are bf16 (weights and x pre-cast on host, w_norm folded into
weight rows); statistics, softmax sums, residual accumulation and PSUM stay
fp32.
"""

import numpy as np
import ml_dtypes

import concourse.bass as bass
import concourse.mybir as mybir
import concourse.tile as tile
from concourse import bacc
from concourse import bass_utils
from concourse.masks import make_identity

AF = mybir.ActivationFunctionType
ALU = mybir.AluOpType
F32 = mybir.dt.float32
BF16 = mybir.dt.bfloat16
MMDT = BF16
NP_MMDT = ml_dtypes.bfloat16

P = 128
T = 2048
C = 2048
D = 128
H = 16
HPC = 2          # heads per core
NB = 2           # batches (all processed on every core)
HID = 5632
HID_T = HID // P  # 44 hid tiles
TQ = 512         # A2A / output col-block granularity
TQA = 1024       # attention query-chunk
EPS = 1e-6
ROPE_BASE = 10000.0
CT = C // P      # 16 contraction tiles
HD2 = D // 2


def _build():
    nc = bacc.Bacc(None, target_bir_lowering=False, num_devices=8)

    # ---- kernel I/O ----
    x_t = nc.dram_tensor("x_t", [C, NB * T], MMDT, kind="ExternalInput")
    wq = nc.dram_tensor("wq", [HPC, P, CT * P], MMDT, kind="ExternalInput")
    wk = nc.dram_tensor("wk", [HPC, P, CT * P], MMDT, kind="ExternalInput")
    wv = nc.dram_tensor("wv", [P, CT, HPC * D], MMDT, kind="ExternalInput")
    wpe_r = nc.dram_tensor("wpe_r", [2, CT, P, 8 * P], MMDT, kind="ExternalInput")
    w1t = nc.dram_tensor("w1t", [HID_T, P, CT * P], MMDT, kind="ExternalInput")
    w2t = nc.dram_tensor("w2t", [HID_T, P, CT * P], MMDT, kind="ExternalInput")
    w3r = nc.dram_tensor("w3r", [CT, P, HID_T * P], MMDT, kind="ExternalInput")
    rope_t = nc.dram_tensor("rope_t", [D, T], F32, kind="ExternalInput")
    tri = nc.dram_tensor("tri", [P, P], MMDT, kind="ExternalInput")
    x_res = nc.dram_tensor("x_res", [C, TQ], MMDT, kind="ExternalInput")
    out = nc.dram_tensor("out", [C, TQ], F32, kind="ExternalOutput")

    inv_sqrt_d = 1.0 / float(np.sqrt(D))
    GROUPS = [[0, 1, 2, 3, 4, 5, 6, 7]]

    with tile.TileContext(nc) as tc:
        with (
            tc.tile_pool(name="const", bufs=1) as const,
            tc.tile_pool(name="dram", bufs=1, space="DRAM") as dram,
        ):
            # ---- constants ----
            ones_f = const.tile([P, 1], F32)
            nc.vector.memset(ones_f, 1.0)
            ones_r = const.tile([P, 1], MMDT)
            nc.vector.tensor_copy(out=ones_r, in_=ones_f)
            eps_sb = const.tile([P, 1], F32)
            nc.vector.memset(eps_sb, EPS)
            ident_f = const.tile([P, P], F32)
            make_identity(nc, ident_f)
            rope_sb = const.tile([D, T], F32)
            tri_sb = const.tile([P, P], MMDT)

            # ---- DRAM scratch for collectives ----
            a2a1_in = dram.tile([8, P, TQ], MMDT)
            a2a1_out = dram.tile([8, P, TQ], MMDT)
            a2a2_in = dram.tile([8, P, TQ], MMDT)
            a2a2_out = dram.tile([8, P, TQ], MMDT)

            # ---- persistent SBUF: first MLP weight tiles (filled during
            # attention while the DMA engines are otherwise idle) ----
            NPRE = 3
            wpre_ctx = tc.tile_pool(name="wpre", bufs=1)
            wpre = wpre_ctx.__enter__()
            w1pre = wpre.tile([P, NPRE, CT, P], MMDT, tag="w1pre", bufs=1)
            w2pre = wpre.tile([P, NPRE, CT, P], MMDT, tag="w2pre", bufs=1)

            # ---- persistent SBUF across phase A (q/k/v resident) ----
            # instance index: inst = 2*m + b  (m: head-in-pair, b: batch)
            qkvp_ctx = tc.tile_pool(name="qkvp", bufs=1)
            qkvp = qkvp_ctx.__enter__()
            qT_sb = qkvp.tile([P, 2 * HPC, T], MMDT, tag="qT_sb", bufs=1)
            kT_sb = qkvp.tile([P, 2 * HPC, T], MMDT, tag="kT_sb", bufs=1)
            v_sb = qkvp.tile([P, NB * T // P, HPC * D], MMDT, tag="v_sb", bufs=1)

            # ================= Phase A1+A2: rmsnorm1 + QKV (chunked) ============
            with (
                tc.tile_pool(name="p12", bufs=2) as p12,
                tc.tile_pool(name="p12psum", bufs=2, space="PSUM") as pp12,
            ):
                CHUNKS = [256, 256, 512, 512, 512]  # per batch (sums to T)
                wq_sb = p12.tile([P, HPC, CT, P], MMDT, tag="wq_sb", bufs=1)
                wk_sb = p12.tile([P, HPC, CT, P], MMDT, tag="wk_sb", bufs=1)
                nc.sync.dma_start(out=wq_sb[:, 0], in_=wq[0])
                first_xt = p12.tile([P, CT, CHUNKS[0]], MMDT, tag="xt256", bufs=2)
                nc.sync.dma_start(
                    out=first_xt,
                    in_=x_t[:, 0 : CHUNKS[0]].rearrange("(ct p) t -> p ct t", p=P),
                )
                for m_ in range(1, HPC):
                    nc.sync.dma_start(out=wq_sb[:, m_], in_=wq[m_])
                for m_ in range(HPC):
                    nc.sync.dma_start(out=wk_sb[:, m_], in_=wk[m_])
                wv_sb = p12.tile([P, CT, HPC * D], MMDT, tag="wv_sb", bufs=1)
                nc.sync.dma_start(out=wv_sb, in_=wv[:, :, :])
                nc.sync.dma_start(out=rope_sb, in_=rope_t[:, :])
                nc.sync.dma_start(out=tri_sb, in_=tri[:, :])

                for b in range(NB):
                    t0p = 0  # within-batch position
                    for ch, CHW in enumerate(CHUNKS):
                        t0 = b * T + t0p  # global column in x_t / v_sb
                        if b == 0 and ch == 0:
                            xt = first_xt
                        else:
                            xt = p12.tile(
                                [P, CT, CHW], MMDT, tag=f"xt{CHW}", bufs=2, name="xt"
                            )
                            nc.sync.dma_start(
                                out=xt,
                                in_=x_t[:, t0 : t0 + CHW].rearrange(
                                    "(ct p) t -> p ct t", p=P
                                ),
                            )
                        # rmsnorm stats: squares on ScalarE, partition-sum on PE
                        sq = p12.tile([P, CT, CHW], MMDT, tag=f"sq{CHW}", bufs=1, name="sq")
                        nc.scalar.activation(
                            sq.rearrange("p a b -> p (a b)"),
                            xt.rearrange("p a b -> p (a b)"),
                            AF.Square,
                        )
                        ssum = pp12.tile([1, TQ], F32, tag="ssum", bufs=1, name="ssum")[:, :CHW]
                        for ct in range(CT):
                            nc.tensor.matmul(
                                ssum,
                                ones_r,
                                sq[:, ct, :],
                                start=(ct == 0),
                                stop=(ct == CT - 1),
                            )
                        srow = p12.tile([1, TQ], F32, tag="srow", bufs=2, name="srow")[:, :CHW]
                        nc.scalar.activation(
                            srow, ssum, AF.Sqrt, bias=eps_sb[0:1, :], scale=1.0 / C
                        )
                        rstd_row = p12.tile([1, TQ], F32, tag="rstd_row", bufs=2, name="rstd_row")[:, :CHW]
                        nc.vector.reciprocal_approx_fast(out=rstd_row, in_=srow)
                        rstd_bc = p12.tile([P, TQ], F32, tag="rstd_bc", bufs=2, name="rstd_bc")[:, :CHW]
                        nc.gpsimd.partition_broadcast(rstd_bc[:], rstd_row[:])
                        # rstd folded into rope tables: rows 0:64 cos*rstd,
                        # 64:128 sin*rstd (q/k matmuls run on RAW x so the PE
                        # never waits on the stats chain)
                        cs_r = p12.tile([P, TQ], F32, tag="cs_r", bufs=2, name="cs_r")[:, :CHW]
                        nc.vector.tensor_tensor(
                            out=cs_r, in0=rope_sb[:, t0p : t0p + CHW], in1=rstd_bc,
                            op=ALU.mult,
                        )

                        # q^T / k^T with fused rope(+rstd) on eviction
                        for w_sb, dst in ((wq_sb, qT_sb), (wk_sb, kT_sb)):
                            for m in range(HPC):
                                inst = 2 * m + b
                                pq = pp12.tile([P, TQ], F32, tag="qk", bufs=4, name="pq")[:, :CHW]
                                for ct in range(CT):
                                    nc.tensor.matmul(
                                        pq,
                                        w_sb[:, m, ct, :],
                                        xt[:, ct, :],
                                        start=(ct == 0),
                                        stop=(ct == CT - 1),
                                    )
                                x1 = pq[0:HD2, :]
                                x2 = pq[HD2:P, :]
                                cosw = cs_r[0:HD2, :]
                                sinw = cs_r[HD2:P, :]
                                tm1 = p12.tile([HD2, TQ], F32, tag="tm1", bufs=2, name="tm1")[:, :CHW]
                                tm2 = p12.tile([HD2, TQ], F32, tag="tm2", bufs=2, name="tm2")[:, :CHW]
                                nc.vector.tensor_tensor(out=tm1, in0=x1, in1=cosw, op=ALU.mult)
                                nc.vector.tensor_tensor(out=tm2, in0=x2, in1=sinw, op=ALU.mult)
                                nc.vector.tensor_tensor(
                                    out=dst[0:HD2, inst, t0p : t0p + CHW],
                                    in0=tm1,
                                    in1=tm2,
                                    op=ALU.subtract,
                                )
                                nc.vector.tensor_tensor(out=tm1, in0=x1, in1=sinw, op=ALU.mult)
                                nc.vector.tensor_tensor(out=tm2, in0=x2, in1=cosw, op=ALU.mult)
                                nc.vector.tensor_tensor(
                                    out=dst[HD2:P, inst, t0p : t0p + CHW],
                                    in0=tm1,
                                    in1=tm2,
                                    op=ALU.add,
                                )

                        # v in row layout [t, 2*D]; per-row rstd via PE-transposed col
                        for rt in range(CHW // P):
                            trp = pp12.tile([P, P], F32, tag="trp", bufs=1)
                            nc.tensor.transpose(
                                trp, rstd_bc[:, rt * P : (rt + 1) * P], ident_f
                            )
                            rstd_col = p12.tile([P, 1], F32, tag="rstd_col", bufs=2)
                            nc.vector.tensor_copy(out=rstd_col, in_=trp[:, 0:1])
                            pv = pp12.tile([P, HPC * D], F32, tag="v", bufs=2)
                            for ct in range(CT):
                                nc.tensor.matmul(
                                    pv,
                                    xt[:, ct, rt * P : (rt + 1) * P],
                                    wv_sb[:, ct, :],
                                    start=(ct == 0),
                                    stop=(ct == CT - 1),
                                )
                            nc.vector.tensor_scalar(
                                out=v_sb[:, t0 // P + rt, :],
                                in0=pv,
                                scalar1=rstd_col,
                                scalar2=None,
                                op0=ALU.mult,
                            )
                        t0p += CHW

            # prefetch the first MLP weight tiles during attention (DMA idle)
            for hp in range(NPRE):
                nc.sync.dma_start(out=w1pre[:, hp], in_=w1t[hp])
            for hp in range(NPRE):
                nc.sync.dma_start(out=w2pre[:, hp], in_=w2t[hp])

            # ================= Phase A3: causal attention (+ A2As) ==============
            with (
                tc.tile_pool(name="att", bufs=2) as att,
                tc.tile_pool(name="attpsum", bufs=2, space="PSUM") as pat,
            ):
                for m in range(HPC):
                    a2a_in = (a2a1_in, a2a2_in)[m]
                    for b in range(NB):
                        inst = 2 * m + b
                        for q2 in range(T // TQA):
                            qb = q2 * TQA
                            o_ps = pat.tile([P, TQA], F32, tag="o", bufs=1)
                            es = []
                            n_items = 8 * q2 + 8
                            # last writer of bank0 (cols 0:TQ) is the r=3 diag
                            # item; last writer of bank1 is the final item
                            b0_last = n_items - 5

                            l_ps = pat.tile([1, TQA], F32, tag="l", bufs=1)
                            l_inv = att.tile([1, TQA], F32, tag="l_inv", bufs=2)
                            l_bc = att.tile([P, TQA], F32, tag="l_bc", bufs=2)
                            oT = att.tile([P, TQA], MMDT, tag="oT", bufs=2)

                            def norm_half(lo, hi, i):
                                # denominator + normalize for one o_ps bank;
                                # bank0's chain runs inside the AV pass
                                nc.tensor.matmul(
                                    l_ps[:, lo:hi], ones_r, eacc[:, lo:hi],
                                    start=True, stop=True,
                                )
                                nc.vector.reciprocal_approx_fast(
                                    out=l_inv[:, lo:hi], in_=l_ps[:, lo:hi]
                                )
                                nc.gpsimd.partition_broadcast(
                                    l_bc[:, lo:hi], l_inv[:, lo:hi]
                                )
                                nc.vector.tensor_tensor(
                                    out=oT[:, lo:hi], in0=o_ps[:, lo:hi],
                                    in1=l_bc[:, lo:hi], op=ALU.mult,
                                )
                                qc = 2 * q2 + i
                                nc.sync.dma_start(
                                    out=a2a_in[b * 4 + qc, :, :],
                                    in_=oT[:, lo:hi],
                                )

                            def av_emit(idx):
                                kb, q0, e = es[idx]
                                first = idx == 0
                                if q0 < TQ:
                                    nc.tensor.matmul(
                                        o_ps[:, q0:TQ],
                                        v_sb[:, b * (T // P) + kb, m * D : (m + 1) * D],
                                        e[:, q0:TQ],
                                        start=first, stop=(idx == b0_last),
                                    )
                                    nc.tensor.matmul(
                                        o_ps[:, TQ:TQA],
                                        v_sb[:, b * (T // P) + kb, m * D : (m + 1) * D],
                                        e[:, TQ:TQA],
                                        start=first, stop=(idx == n_items - 1),
                                    )
                                else:
                                    nc.tensor.matmul(
                                        o_ps[:, q0:TQA],
                                        v_sb[:, b * (T // P) + kb, m * D : (m + 1) * D],
                                        e[:, q0:TQA],
                                        start=first, stop=(idx == n_items - 1),
                                    )
                                if idx == b0_last:
                                    norm_half(0, TQ, 0)
                            # full key blocks
                            for kb in range(8 * q2):
                                st = pat.tile([P, TQA], F32, tag="st", bufs=2)
                                for i in range(2):
                                    nc.tensor.matmul(
                                        st[:, i * TQ : (i + 1) * TQ],
                                        kT_sb[:, inst, kb * P : (kb + 1) * P],
                                        qT_sb[:, inst, qb + i * TQ : qb + (i + 1) * TQ],
                                        start=True,
                                        stop=True,
                                    )
                                e = att.tile([P, TQA], MMDT, tag="e", bufs=18)
                                nc.scalar.activation(e, st, AF.Exp, scale=inv_sqrt_d)
                                if not es:
                                    eacc = att.tile([P, TQA], MMDT, tag="eacc", bufs=2)
                                    nc.vector.tensor_copy(out=eacc, in_=e)
                                else:
                                    nc.vector.tensor_tensor(
                                        out=eacc, in0=eacc, in1=e, op=ALU.add
                                    )
                                es.append((kb, 0, e))
                                if len(es) >= 6:
                                    av_emit(len(es) - 6)
                            # diagonal blocks (r = 0..7), masked region trimmed
                            for r in range(8):
                                kb = 8 * q2 + r
                                q0 = r * P
                                st = pat.tile([P, TQA], F32, tag="st", bufs=2)
                                if q0 < TQ:
                                    nc.tensor.matmul(
                                        st[:, q0:TQ],
                                        kT_sb[:, inst, kb * P : (kb + 1) * P],
                                        qT_sb[:, inst, qb + q0 : qb + TQ],
                                        start=True,
                                        stop=True,
                                    )
                                    nc.tensor.matmul(
                                        st[:, TQ:TQA],
                                        kT_sb[:, inst, kb * P : (kb + 1) * P],
                                        qT_sb[:, inst, qb + TQ : qb + TQA],
                                        start=True,
                                        stop=True,
                                    )
                                else:
                                    nc.tensor.matmul(
                                        st[:, q0:TQA],
                                        kT_sb[:, inst, kb * P : (kb + 1) * P],
                                        qT_sb[:, inst, qb + q0 : qb + TQA],
                                        start=True,
                                        stop=True,
                                    )
                                e = att.tile([P, TQA], MMDT, tag="e", bufs=18)
                                nc.scalar.activation(
                                    e[:, q0:TQA], st[:, q0:TQA], AF.Exp, scale=inv_sqrt_d
                                )
                                nc.vector.tensor_tensor(
                                    out=e[:, q0 : q0 + P],
                                    in0=e[:, q0 : q0 + P],
                                    in1=tri_sb,
                                    op=ALU.mult,
                                )
                                if not es:
                                    eacc = att.tile([P, TQA], MMDT, tag="eacc", bufs=2)
                                    nc.vector.tensor_copy(out=eacc, in_=e)
                                else:
                                    nc.vector.tensor_tensor(
                                        out=eacc[:, q0:TQA],
                                        in0=eacc[:, q0:TQA],
                                        in1=e[:, q0:TQA],
                                        op=ALU.add,
                                    )
                                es.append((kb, q0, e))
                                if len(es) >= 6:
                                    av_emit(len(es) - 6)
                            # drain the AV pipeline; bank1's normalize chain
                            # then overlaps the next group's QK pass
                            for idx in range(n_items - 5, n_items):
                                av_emit(idx)
                            norm_half(TQ, TQA, 1)
                    if m == 0:
                        nc.gpsimd.collective_compute(
                            "AllToAll",
                            ALU.bypass,
                            replica_groups=GROUPS,
                            ins=[a2a1_in.opt()],
                            outs=[a2a1_out.opt()],
                        )
            qkvp_ctx.__exit__(None, None, None)

            # A2A2 emitted outside the attention pool so its completion doesn't
            # gate the pool-close barrier; it overlaps proj pass 0.
            nc.gpsimd.collective_compute(
                "AllToAll",
                ALU.bypass,
                replica_groups=GROUPS,
                ins=[a2a2_in.opt()],
                outs=[a2a2_out.opt()],
            )

            # ---- persistent SBUF through phase B ----
            bper_ctx = tc.tile_pool(name="bper", bufs=1)
            bper = bper_ctx.__enter__()
            xmidT = bper.tile([P, CT, TQ], F32, tag="xmidT", bufs=1)
            h2T = bper.tile([P, CT, TQ], MMDT, tag="h2T", bufs=1)

            # ========== Phase B1: proj^T + residual + rmsnorm2 (transposed) ======
            with (
                tc.tile_pool(name="proj", bufs=2) as prj,
                tc.tile_pool(name="projpsum", bufs=2, space="PSUM") as ppj,
            ):
                lp0 = prj.tile([P, 8, TQ], MMDT, tag="lp0", bufs=1)
                lp1 = prj.tile([P, 8, TQ], MMDT, tag="lp1", bufs=1)
                wpe_first = prj.tile([P, 8, P], MMDT, tag="wpe_first", bufs=1)
                nc.sync.dma_start(out=wpe_first, in_=wpe_r[0, 0, :, 0 : 8 * P])
                # lp0[p, s, t] = a2a1_out[s, p, t]: slot s = head 2s, one DMA.
                nc.sync.dma_start(
                    out=lp0, in_=a2a1_out.rearrange("s p t -> p s t")
                )
                # pass 0: even heads (a2a1), into xmidT
                for ct in range(CT):
                    if ct == 0:
                        wpe_sb = wpe_first
                    else:
                        wpe_sb = prj.tile([P, 8, P], MMDT, tag="wpe_sb", bufs=3)
                        nc.sync.dma_start(out=wpe_sb, in_=wpe_r[0, ct, :, 0 : 8 * P])
                    yps = ppj.tile([P, TQ], F32, tag="y", bufs=4)
                    for blk in range(8):
                        nc.tensor.matmul(
                            yps,
                            wpe_sb[:, blk, :],
                            lp0[:, blk, :],
                            start=(blk == 0),
                            stop=(blk == 7),
                        )
                    nc.scalar.copy(out=xmidT[:, ct, :], in_=yps)
                # residual slice of x (bf16, per-core input)
                xres = prj.tile([P, CT, TQ], MMDT, tag="xres", bufs=1)
                nc.sync.dma_start(
                    out=xres,
                    in_=x_res[:, :].rearrange("(ct p) t -> p ct t", p=P),
                )
                # pass 1: odd heads (a2a2) + residual, rmsnorm2 stats per ct
                nc.sync.dma_start(
                    out=lp1, in_=a2a2_out.rearrange("s p t -> p s t")
                )
                sq2 = bper.tile([P, CT, TQ], MMDT, tag="sq2", bufs=1)
                ssum2 = ppj.tile([1, TQ], F32, tag="ssum2", bufs=1)
                for ct in range(CT):
                    wpe_sb1 = prj.tile([P, 8, P], MMDT, tag="wpe_sb1", bufs=3)
                    nc.sync.dma_start(
                        out=wpe_sb1, in_=wpe_r[1, ct, :, 0 : 8 * P]
                    )
                    yps = ppj.tile([P, TQ], F32, tag="y", bufs=4)
                    for blk in range(8):
                        nc.tensor.matmul(
                            yps,
                            wpe_sb1[:, blk, :],
                            lp1[:, blk, :],
                            start=(blk == 0),
                            stop=(blk == 7),
                        )
                    t1 = prj.tile([P, TQ], F32, tag="t1", bufs=3)
                    nc.vector.tensor_tensor(
                        out=t1, in0=yps, in1=xmidT[:, ct, :], op=ALU.add
                    )
                    nc.vector.tensor_tensor(
                        out=xmidT[:, ct, :], in0=t1, in1=xres[:, ct, :], op=ALU.add
                    )
                    nc.scalar.activation(
                        sq2[:, ct, :], xmidT[:, ct, :], AF.Square
                    )
                    nc.tensor.matmul(
                        ssum2, ones_r, sq2[:, ct, :], start=(ct == 0), stop=(ct == CT - 1)
                    )
                srow2 = bper.tile([1, TQ], F32, tag="srow2", bufs=1)
                nc.scalar.activation(
                    srow2, ssum2, AF.Sqrt, bias=eps_sb[0:1, :], scale=1.0 / C
                )
                rstd2 = bper.tile([1, TQ], F32, tag="rstd2", bufs=1)
                nc.vector.reciprocal_approx_fast(out=rstd2, in_=srow2)
                rstd2_bc = bper.tile([P, TQ], F32, tag="rstd2_bc", bufs=1)
                nc.gpsimd.partition_broadcast(rstd2_bc[:], rstd2[:])
                for ct in range(CT):
                    nc.vector.tensor_tensor(
                        out=h2T[:, ct, :], in0=xmidT[:, ct, :], in1=rstd2_bc, op=ALU.mult
                    )

            # ================= Phase B2: SwiGLU (transposed w3 pass) =============
            with (
                tc.tile_pool(name="mlp", bufs=2) as mlp,
                tc.tile_pool(name="mlppsum", bufs=2, space="PSUM") as pml,
            ):
                uT = mlp.tile([P, HID_T, TQ], MMDT, tag="uT", bufs=1)
                for ht in range(HID_T):
                    if ht < NPRE:
                        w1_sb = w1pre[:, ht]
                        w2_sb = w2pre[:, ht]
                    else:
                        w1_sb = mlp.tile([P, CT, P], MMDT, tag="w1_sb", bufs=3)
                        nc.sync.dma_start(out=w1_sb, in_=w1t[ht])
                        w2_sb = mlp.tile([P, CT, P], MMDT, tag="w2_sb", bufs=3)
                        nc.sync.dma_start(out=w2_sb, in_=w2t[ht])
                    g1 = pml.tile([P, TQ], F32, tag="g1", bufs=2)
                    g2 = pml.tile([P, TQ], F32, tag="g2", bufs=2)
                    for ct in range(CT):
                        nc.tensor.matmul(
                            g1, w1_sb[:, ct, :], h2T[:, ct, :],
                            start=(ct == 0), stop=(ct == CT - 1),
                        )
                    for ct in range(CT):
                        nc.tensor.matmul(
                            g2, w2_sb[:, ct, :], h2T[:, ct, :],
                            start=(ct == 0), stop=(ct == CT - 1),
                        )
                    sil = mlp.tile([P, TQ], F32, tag="sil", bufs=3)
                    nc.scalar.activation(sil, g1, AF.Silu)
                    nc.vector.tensor_tensor(
                        out=uT[:, ht, :], in0=g2, in1=sil, op=ALU.mult
                    )
                # y3^T: stationary w3 blocks, moving uT; accumulate 44 ht per ct
                for ct in range(CT):
                    w3_sb = mlp.tile([P, HID_T, P], MMDT, tag="w3_sb", bufs=2)
                    nc.sync.dma_start(out=w3_sb, in_=w3r[ct])
                    y3 = pml.tile([P, TQ], F32, tag="y3", bufs=2)
                    for ht in range(HID_T):
                        nc.tensor.matmul(
                            y3, w3_sb[:, ht, :], uT[:, ht, :],
                            start=(ht == 0), stop=(ht == HID_T - 1),
                        )
                    ofin = mlp.tile([P, TQ], F32, tag="ofin", bufs=3)
                    nc.vector.tensor_tensor(
                        out=ofin, in0=y3, in1=xmidT[:, ct, :], op=ALU.add
                    )
                    nc.sync.dma_start(out=out[ct * P : (ct + 1) * P, :], in_=ofin)
            bper_ctx.__exit__(None, None, None)
            wpre_ctx.__exit__(None, None, None)

    nc.compile()
    return nc


_NC_CACHE = None


def _get_nc():
    global _NC_CACHE
    if _NC_CACHE is None:
        _NC_CACHE = _build()
    return _NC_CACHE


def _host_inputs(x, w_norm1, w_qkv, w_proj, w_norm2, w1, w2, w3):
    x = np.asarray(x, dtype=np.float32)
    w_qkv = np.asarray(w_qkv, dtype=np.float32)
    w_proj = np.asarray(w_proj, dtype=np.float32)
    w_norm1 = np.asarray(w_norm1, dtype=np.float32)
    w_norm2 = np.asarray(w_norm2, dtype=np.float32)
    w1 = np.asarray(w1, dtype=np.float32)
    w2 = np.asarray(w2, dtype=np.float32)
    w3 = np.asarray(w3, dtype=np.float32)

    half = D // 2
    inv_freq = 1.0 / (ROPE_BASE ** (np.arange(half, dtype=np.float32) / half))
    pos = np.arange(T, dtype=np.float32)
    freqs = pos[:, None] * inv_freq[None, :]
    rope_tab = np.ascontiguousarray(
        np.concatenate([np.cos(freqs).T, np.sin(freqs).T], axis=0).astype(np.float32)
    )

    ql = np.arange(P)[None, :]
    kv = np.arange(P)[:, None]
    tri = (ql >= kv).astype(NP_MMDT)

    # fold w_norm into weight rows (h @ W == (x*rstd) @ (diag(wn) W))
    w_qkv_n = w_qkv * w_norm1[:, None]
    w1_n = w1 * w_norm2[:, None]
    w2_n = w2 * w_norm2[:, None]

    # [HID_T, P, CT*P]: w1t[ht, p, ct*P + d] = w1_n[ct*P + p, ht*P + d]
    w1t = np.ascontiguousarray(
        w1_n.reshape(CT, P, HID_T, P).transpose(2, 1, 0, 3).reshape(HID_T, P, C)
    ).astype(NP_MMDT)
    w2t = np.ascontiguousarray(
        w2_n.reshape(CT, P, HID_T, P).transpose(2, 1, 0, 3).reshape(HID_T, P, C)
    ).astype(NP_MMDT)
    # [CT, P, HID_T*P]: w3r[ct, p, ht*P + d] = w3[ht*P + p, ct*P + d]
    w3r_h = np.ascontiguousarray(
        w3.reshape(HID_T, P, CT, P).transpose(2, 1, 0, 3).reshape(CT, P, HID)
    ).astype(NP_MMDT)

    # [P, CT, cols]: wqkv_r[p, ct, d] = w_qkv_n[ct*P + p, d]
    wqkv_r = np.ascontiguousarray(
        w_qkv_n.reshape(CT, P, 3 * C).transpose(1, 0, 2)
    ).astype(NP_MMDT)

    # wpe: [2, CT, P, 8*P].  Pass p block s = w_proj rows of head (2s + p)
    # (receiver slot s carries head 2s for pass 0, head 2s+1 for pass 1).
    wpe_full = np.empty((2, 8, P, C), dtype=np.float32)
    for s_ in range(8):
        wpe_full[0, s_] = w_proj[(2 * s_) * P : (2 * s_ + 1) * P, :]
        wpe_full[1, s_] = w_proj[(2 * s_ + 1) * P : (2 * s_ + 2) * P, :]
    wpe_r_h = np.ascontiguousarray(
        wpe_full.reshape(2, 8, P, CT, P).transpose(0, 3, 2, 1, 4).reshape(2, CT, P, 8 * P)
    ).astype(NP_MMDT)

    # x_t: [C, 2T] both batches, bf16 (shared across cores)
    x_t_h = np.ascontiguousarray(
        np.concatenate([x[0].T, x[1].T], axis=1)
    ).astype(NP_MMDT)

    in_maps = []
    for j in range(8):
        b, hg = j // 4, j % 4
        col0 = 2 * j * D  # first head of this core's pair
        in_maps.append(
            {
                "x_t": x_t_h,
                "wq": np.ascontiguousarray(
                    wqkv_r[:, :, col0 : col0 + HPC * D]
                    .reshape(P, CT, HPC, P)
                    .transpose(2, 0, 1, 3)
                    .reshape(HPC, P, C)
                ),
                "wk": np.ascontiguousarray(
                    wqkv_r[:, :, C + col0 : C + col0 + HPC * D]
                    .reshape(P, CT, HPC, P)
                    .transpose(2, 0, 1, 3)
                    .reshape(HPC, P, C)
                ),
                "wv": np.ascontiguousarray(
                    wqkv_r[:, :, 2 * C + col0 : 2 * C + col0 + HPC * D]
                ),
                "wpe_r": wpe_r_h,
                "w1t": w1t,
                "w2t": w2t,
                "w3r": w3r_h,
                "rope_t": rope_tab,
                "tri": tri,
                "x_res": np.ascontiguousarray(
                    x_t_h[:, b * T + hg * TQ : b * T + (hg + 1) * TQ]
                ),
            }
        )
    return in_maps


def kernel(x, w_norm1, w_qkv, w_proj, w_norm2, w1, w2, w3, _trace=False, _tmpdir=None):
    nc = _get_nc()
    in_maps = _host_inputs(x, w_norm1, w_qkv, w_proj, w_norm2, w1, w2, w3)
    kwargs = {}
    if _trace:
        kwargs = {"trace": True, "tmpdir": _tmpdir}
    res = bass_utils.run_bass_kernel_spmd(
        nc, in_maps, core_ids=list(range(8)), **kwargs
    )
    out = np.empty((2, T, C), dtype=np.float32)
    for j in range(8):
        out[j // 4, (j % 4) * TQ : (j % 4 + 1) * TQ, :] = res.results[j]["out"].T
    kernel._last_exec_time_ns = res.exec_time_ns
    return out
